# revision 30
# baseline (speedup 1.0000x reference)
"""Trainium2 Bass kernel for nn_CategoryInterestAttention.

Contract: kernel(**inputs) takes FULL unsharded inputs (as produced by the
problem's setup_inputs) and returns the FULL (512, 128) float32 output.

Strategy (pure data parallel, 8 NeuronCores, 64 batch rows each):
  - Categories are indexed by VALUE g in [0, 64) instead of the reference's
    sorted-unique slot index; softmax over present groups is permutation
    invariant so the final output is identical.
  - match[g,t] = (cat[t]==g)&mask[t]; query of group g = seq embedding of the
    LAST matching position (argmax of (t+1)*match), gathered by indirect DMA.
  - LayerNorm gains/biases are folded into the projection weights host-side;
    the normalized sequence z=(seq-mu)/sigma is computed once on-chip and
    shared by both layers. The v bias is folded into the wo bias (attention
    weights sum to 1 per group).
  - bf16 matmuls with fp32 PSUM accumulation; the final target-attention
    stage runs in fp32 (it dominates the error budget).
  - Attention per (row, t-chunk) in a (t, g) layout:
      scoresT  = kT_chunk.T @ q_blocked + (BIG/s)*match  (two matmuls into
                 one PSUM tile; q head-block-diagonal => 4 heads at once)
      E        = exp(s*scoresT - BIG)     (masking folded into the exp bias;
                                           non-matching entries ~1e-13)
      ctx|den  = E_hh.T @ [v_h0|1|v_h1|1] (ones columns give the softmax
                                           denominators for free)
  - All transposes on the PE (DMA transpose measured ~1.2us each on the sync
    engine); attention/FFN outputs are transposed into PSUM and added to the
    fp32 token-major residual directly from PSUM.
"""

import numpy as np
import ml_dtypes

import concourse.bass as bass
import concourse.bacc as bacc
import concourse.tile as tile
from concourse import mybir
from concourse.bass_utils import run_bass_kernel_spmd

F32 = mybir.dt.float32
BF16 = mybir.dt.bfloat16
I32 = mybir.dt.int32
AF = mybir.ActivationFunctionType
OP = mybir.AluOpType

B, T, D = 512, 256, 128
C, H, L, F = 64, 4, 2, 512
HD = D // H                    # 32
NCORES = 8
R = B // NCORES                # 64 rows per core
NT = R * T                     # 16384 seq tokens per core
NX = R * C                     # 4096 group tokens per core
NTC = NT // 128                # 128 seq chunks
NXT = NX // 128                # 32 x-token tiles
SCALE_S = 1.0 / np.sqrt(np.float32(HD))
SCALE_L = 1.0 / np.sqrt(np.float32(D))
EPS = 1e-5
BIG = 30.0                     # mask offset: exp(-30) ~ 9e-14


def _build(nc):
    seq = nc.dram_tensor("seq", [NT, D], F32, kind="ExternalInput").ap()
    seqb = nc.dram_tensor("seqb", [NT, D], BF16, kind="ExternalInput").ap()
    catm = nc.dram_tensor("catm", [R, T], mybir.dt.int8,
                          kind="ExternalInput").ap()
    tgt = nc.dram_tensor("tgt", [R, D], F32, kind="ExternalInput").ap()
    iota_g2 = nc.dram_tensor("iota_g2", [128, 1], F32, kind="ExternalInput").ap()
    iota_t = nc.dram_tensor("iota_t", [128, T], F32, kind="ExternalInput").ap()
    rowbase = nc.dram_tensor("rowbase", [128, NXT], F32, kind="ExternalInput").ap()
    ident = nc.dram_tensor("ident", [128, 128], F32, kind="ExternalInput").ap()
    identb = nc.dram_tensor("identb", [128, 128], BF16, kind="ExternalInput").ap()
    iota_gr = nc.dram_tensor("iota_gr", [128, C], F32, kind="ExternalInput").ap()
    io = dict(seq=seq, seqb=seqb, catm=catm, tgt=tgt, iota_g2=iota_g2,
              iota_t=iota_t, rowbase=rowbase, ident=ident, identb=identb,
              iota_gr=iota_gr)
    for name, shape, dt_ in [
        ("wkt", [D, D], BF16), ("wvt", [D, D], BF16), ("wqt", [D, D], BF16),
        ("wot", [D, D], BF16), ("w1t", [D, F], BF16), ("w2t", [D, F], BF16),
        ("bk", [D, 1], F32), ("bq", [D, 1], F32), ("bo", [D, 1], F32),
        ("b2", [D, 1], F32), ("b1_", [128, 4], F32),
    ]:
        io[name] = [nc.dram_tensor(f"{name}{l}", shape, dt_,
                                   kind="ExternalInput").ap() for l in range(L)]
    io["out"] = nc.dram_tensor("out", [R, D], F32, kind="ExternalOutput").ap()

    with tile.TileContext(nc) as tc:
        from contextlib import ExitStack
        with ExitStack() as ctx:
            _body(ctx, tc, nc, io)
    return nc


def _body(ctx, tc, nc, io):
    P = 128
    persist = ctx.enter_context(tc.tile_pool(name="persist", bufs=1))
    consts = ctx.enter_context(tc.tile_pool(name="consts", bufs=1))
    ld = ctx.enter_context(tc.tile_pool(name="ld", bufs=4))
    small = ctx.enter_context(tc.tile_pool(name="small", bufs=4))
    ev = ctx.enter_context(tc.tile_pool(name="ev", bufs=4))
    ctokp = ctx.enter_context(tc.tile_pool(name="ctokp", bufs=6))
    epool = ctx.enter_context(tc.tile_pool(name="epool", bufs=4))
    psA = ctx.enter_context(tc.tile_pool(name="psA", bufs=3, space="PSUM"))
    psC = ctx.enter_context(tc.tile_pool(name="psC", bufs=3, space="PSUM"))
    psT = ctx.enter_context(tc.tile_pool(name="psT", bufs=2, space="PSUM"))
    psX = psC

    # ---- constants ----
    c_ig2 = consts.tile([P, 1], F32)
    nc.sync.dma_start(out=c_ig2, in_=io["iota_g2"])
    c_it = consts.tile([P, T], F32)
    nc.sync.dma_start(out=c_it, in_=io["iota_t"])
    c_rb = consts.tile([P, NXT], F32)
    nc.sync.dma_start(out=c_rb, in_=io["rowbase"])
    c_id = consts.tile([P, P], F32)
    nc.sync.dma_start(out=c_id, in_=io["ident"])
    c_idb = consts.tile([P, P], BF16)
    nc.sync.dma_start(out=c_idb, in_=io["identb"])
    c_igr = consts.tile([P, C], F32)
    nc.sync.dma_start(out=c_igr, in_=io["iota_gr"])
    c_eps = consts.tile([P, 1], F32)
    nc.vector.memset(c_eps, EPS)
    w = {}
    for name in ("wkt", "wvt", "wqt", "wot", "w1t", "w2t"):
        w[name] = []
        for l in range(L):
            t_ = consts.tile(list(io[name][l].shape), BF16, tag=f"{name}{l}")
            nc.sync.dma_start(out=t_, in_=io[name][l])
            w[name].append(t_)
    bias = {}
    for name in ("bk", "bq", "bo", "b2", "b1_"):
        bias[name] = []
        for l in range(L):
            t_ = consts.tile(list(io[name][l].shape), F32, tag=f"{name}{l}")
            nc.sync.dma_start(out=t_, in_=io[name][l])
            bias[name].append(t_)

    def pe_transpose_b(dst_sbuf_slice, src_tile, ev_engine=None):
        """bf16 (128,128) transpose via PE + evict into an SBUF slice."""
        pt = psT.tile([P, P], BF16, tag="pt")
        nc.tensor.transpose(out=pt, in_=src_tile, identity=c_idb[:])
        (ev_engine or nc.scalar.copy)(out=dst_sbuf_slice, in_=pt)

    # ---- stage 0: z = LN(seq) token-major -> zT feature-major (bf16) ----
    zT = persist.tile([P, NT], BF16)
    for g4 in range(NT // 512):
        s4 = ld.tile([P, 512], BF16, tag="seqld", bufs=2)
        nc.sync.dma_start(out=s4, in_=bass.AP(
            tensor=io["seqb"].tensor, offset=g4 * 512 * D,
            ap=[[D, 128], [128 * D, 4], [1, D]]))
        mvb = small.tile([P, 4, 2], F32, tag="mvb")
        for j in range(4):
            st = small.tile([P, 6], F32, tag="bnst")
            nc.vector.bn_stats(out=st, in_=s4[:, j * 128:(j + 1) * 128])
            nc.vector.bn_aggr(out=mvb[:, j, :], in_=st)
        nc.scalar.activation(out=mvb[:, :, 1:2], in_=mvb[:, :, 1:2],
                             func=AF.Sqrt, bias=c_eps[:])
        nc.vector.reciprocal(out=mvb[:, :, 1:2], in_=mvb[:, :, 1:2])
        nbias = small.tile([P, 4, 1], F32, tag="nbias")
        nc.vector.scalar_tensor_tensor(out=nbias, in0=mvb[:, :, 0:1],
                                       scalar=-1.0, in1=mvb[:, :, 1:2],
                                       op0=OP.mult, op1=OP.mult)
        z4 = ld.tile([P, 512], BF16, tag="ztok")
        for j in range(4):
            nc.scalar.activation(out=z4[:, j * 128:(j + 1) * 128],
                                 in_=s4[:, j * 128:(j + 1) * 128],
                                 func=AF.Identity, bias=nbias[:, j, :],
                                 scale=mvb[:, j, 1:2])
            pe_transpose_b(zT[:, g4 * 512 + j * 128:g4 * 512 + (j + 1) * 128],
                           z4[:, j * 128:(j + 1) * 128],
                           ev_engine=nc.vector.tensor_copy)

    # ---- stage 1: match (g-layout), qidx, present per row-pair ----
    presentf = persist.tile([P, NXT], F32)
    qposf = persist.tile([P, NXT], F32)
    for rp in range(NXT):
        cat_bc = ld.tile([P, T], mybir.dt.int8, tag="catbc", bufs=2)
        for half in range(2):
            r = 2 * rp + half
            nc.gpsimd.dma_start(out=cat_bc[64 * half:64 * half + 64, :],
                                in_=bass.AP(tensor=io["catm"].tensor,
                                            offset=r * T, ap=[[0, 64], [1, T]]))
        catf_bc = small.tile([P, T], F32, tag="catf", bufs=2)
        nc.gpsimd.tensor_copy(out=catf_bc, in_=cat_bc)
        mg = small.tile([P, T], BF16, tag="mg", bufs=2)
        nc.vector.tensor_scalar(out=mg, in0=catf_bc, scalar1=c_ig2,
                                scalar2=None, op0=OP.is_equal)
        nc.vector.tensor_reduce(out=presentf[:, rp:rp + 1], in_=mg,
                                axis=mybir.AxisListType.X, op=OP.max)
        posm = small.tile([P, T], F32, tag="posm", bufs=2)
        nc.gpsimd.tensor_tensor(out=posm, in0=mg, in1=c_it, op=OP.mult)
        nc.vector.tensor_reduce(out=qposf[:, rp:rp + 1], in_=posm,
                                axis=mybir.AxisListType.X, op=OP.max)
    qidx_i = persist.tile([P, NXT], I32)
    tmpq = small.tile([P, NXT], F32, tag="tmpq")
    nc.vector.tensor_scalar(out=tmpq, in0=qposf, scalar1=-1.0, scalar2=0.0,
                            op0=OP.add, op1=OP.max)
    nc.vector.tensor_tensor(out=tmpq, in0=tmpq, in1=c_rb, op=OP.add)
    nc.vector.tensor_copy(out=qidx_i, in_=tmpq)
    pen_tok = persist.tile([P, NXT], F32)
    nc.vector.tensor_scalar(out=pen_tok, in0=presentf, scalar1=-1.0, scalar2=1e9,
                            op0=OP.add, op1=OP.mult)

    # ---- attention match tiles in (t, g) layout, shared by both layers ----
    cat_tok8 = persist.tile([P, NTC], mybir.dt.int8)
    nc.sync.dma_start(out=cat_tok8, in_=bass.AP(
        tensor=io["catm"].tensor, offset=0, ap=[[1, 128], [T, R], [128, 2]]))
    cat_tok = persist.tile([P, NTC], F32)
    nc.vector.tensor_copy(out=cat_tok, in_=cat_tok8)
    m_tg = [persist.tile([P, 2, 1, C], BF16, tag=f"mtg{r}", name=f"mtg{r}")
            for r in range(R)]
    for r in range(R):
        for c in range(2):
            nc.vector.tensor_scalar(out=m_tg[r][:, c, 0, :], in0=c_igr,
                                    scalar1=cat_tok[:, 2 * r + c:2 * r + c + 1],
                                    scalar2=None, op0=OP.is_equal)

    # ---- x0 gather (token-major fp32 master copy of x) ----
    x_f = [persist.tile([P, D], F32, tag=f"x{j}", name=f"x{j}")
           for j in range(NXT)]
    for j in range(NXT):
        nc.gpsimd.indirect_dma_start(
            out=x_f[j][:], out_offset=None, in_=io["seq"][:],
            in_offset=bass.IndirectOffsetOnAxis(ap=qidx_i[:, j:j + 1], axis=0))

    # ---- per-layer persistent buffers ----
    kT = persist.tile([P, NT], BF16)
    # v[cc]: (128, 4, 33) = [v_h | 1] per head (ones column -> denominators)
    v_sb = [persist.tile([P, H, HD + 1], BF16, tag=f"v{cc}", name=f"v{cc}")
            for cc in range(NTC)]
    for cc in range(NTC):
        nc.vector.memset(v_sb[cc][:, :, HD:HD + 1], 1.0)
    xnT = persist.tile([P, NX], BF16)
    # qb slabs: per 8 rows, q in head-block-diagonal layout (zeros elsewhere
    # memset once; the q evictions always overwrite the same block positions)
    qbs = [persist.tile([P, 8 * H * C], BF16, tag=f"qbs{i}", name=f"qbs{i}")
           for i in range(2)]
    for i in range(2):
        nc.vector.memset(qbs[i], 0.0)

    def ln_to(dst_T):
        for g4 in range(NXT // 4):
            mvb = small.tile([P, 4, 2], F32, tag="mvb")
            for j in range(4):
                st = small.tile([P, 6], F32, tag="bnst")
                nc.vector.bn_stats(out=st, in_=x_f[4 * g4 + j])
                nc.vector.bn_aggr(out=mvb[:, j, :], in_=st)
            nc.scalar.activation(out=mvb[:, :, 1:2], in_=mvb[:, :, 1:2],
                                 func=AF.Sqrt, bias=c_eps[:])
            nc.vector.reciprocal(out=mvb[:, :, 1:2], in_=mvb[:, :, 1:2])
            for j in range(4):
                zx = ld.tile([P, D], BF16, tag="zxtok")
                nc.vector.tensor_scalar(out=zx, in0=x_f[4 * g4 + j],
                                        scalar1=mvb[:, j, 0:1],
                                        scalar2=mvb[:, j, 1:2],
                                        op0=OP.subtract, op1=OP.mult)
                pe_transpose_b(dst_T[:, (4 * g4 + j) * 128:(4 * g4 + j + 1) * 128],
                               zx, ev_engine=nc.vector.tensor_copy)

    for l in range(L):
        # ---- kT = Wk' @ z (feature-major), bias via ACT evict ----
        for nn in range(NT // 512):
            ps = psA.tile([P, 512], F32, tag="mm")
            nc.tensor.matmul(out=ps, lhsT=w["wkt"][l][:],
                             rhs=zT[:, nn * 512:(nn + 1) * 512],
                             start=True, stop=True)
            nc.scalar.activation(out=kT[:, nn * 512:(nn + 1) * 512], in_=ps,
                                 func=AF.Identity, bias=bias["bk"][l][:])
        # ---- v token-major, head-pair layout with ones columns ----
        for cc in range(NTC):
            ps = psA.tile([P, D], F32, tag="mm")
            nc.tensor.matmul(out=ps, lhsT=zT[:, cc * 128:(cc + 1) * 128],
                             rhs=w["wvt"][l][:], start=True, stop=True)
            nc.scalar.copy(
                out=v_sb[cc][:, :, 0:HD],
                in_=ps[:].rearrange("p (h c) -> p h c", h=H))
        # ---- attention; 8 rows (one 512-token slab) at a time ----
        ln_to(xnT)
        for sl in range(NX // 512):
            # q for this slab's 8 rows, evicted into block-diagonal layout
            qsl = qbs[sl % 2]
            ps = psA.tile([P, 512], F32, tag="mm")
            nc.tensor.matmul(out=ps, lhsT=w["wqt"][l][:],
                             rhs=xnT[:, sl * 512:(sl + 1) * 512],
                             start=True, stop=True)
            for h in range(H):
                nc.vector.tensor_scalar(
                    out=qsl[HD * h:HD * (h + 1), :].rearrange(
                        "p (rl q) -> p rl q", q=H * C)[:, :, C * h:C * (h + 1)],
                    in0=ps[HD * h:HD * (h + 1), :].rearrange(
                        "p (rl g) -> p rl g", g=C),
                    scalar1=bias["bq"][l][HD * h:HD * (h + 1), :],
                    scalar2=None, op0=OP.add)
            ctx_tok = []
            for rp2 in range(4):           # row pairs within slab
                psc = psC.tile([P, H, HD + 1], F32, tag="ctx")
                for par in range(2):
                    r = 8 * sl + 2 * rp2 + par
                    rl = 2 * rp2 + par
                    qb = qsl[:, rl * H * C:(rl + 1) * H * C]
                    ps = psA.tile([P, 2 * H * C], F32, tag="mm")
                    for c in range(2):
                        cc = 2 * r + c
                        nc.tensor.matmul(out=ps[:, 256 * c:256 * (c + 1)],
                                         lhsT=kT[:, cc * 128:(cc + 1) * 128],
                                         rhs=qb, start=(c == 0), stop=(c == 1))
                    et = epool.tile([P, 2 * H * C], BF16, tag="et")
                    nc.scalar.activation(out=et, in_=ps, func=AF.Exp,
                                         scale=float(SCALE_S))
                    E2 = epool.tile([P, 2 * H * C], BF16, tag="E")
                    nc.vector.tensor_tensor(
                        out=E2[:].rearrange("p (c h g) -> p c h g", c=2, h=H),
                        in0=et[:].rearrange("p (c h g) -> p c h g", c=2, h=H),
                        in1=m_tg[r].to_broadcast([P, 2, H, C]), op=OP.mult)
                    off = 64 * par
                    for h in range(H):
                        for c in range(2):
                            nc.tensor.matmul(
                                out=psc[off:off + 64, h, :],
                                lhsT=E2[:, 256 * c + C * h:256 * c + C * (h + 1)],
                                rhs=v_sb[2 * r + c][:, h, :],
                                start=(h == 0 and c == 0),
                                stop=(h == H - 1 and c == 1))
                rd = small.tile([P, H, 1], F32, tag="rd")
                nc.vector.tensor_scalar(out=rd, in0=psc[:, :, HD:HD + 1],
                                        scalar1=1e-30, scalar2=None, op0=OP.add)
                nc.vector.reciprocal(out=rd, in_=rd)
                ct = ctokp.tile([P, D], BF16, tag="ctok")
                nc.vector.scalar_tensor_tensor(
                    out=ct[:].rearrange("p (h c) -> p h c", h=H),
                    in0=psc[:, :, 0:HD], scalar=1.0,
                    in1=rd.to_broadcast([P, H, HD]),
                    op0=OP.mult, op1=OP.mult)
                ctx_tok.append(ct)
            cT = ev.tile([P, 512], BF16, tag="cT")
            for k in range(4):
                pe_transpose_b(cT[:, k * 128:(k + 1) * 128], ctx_tok[k],
                               ev_engine=nc.vector.tensor_copy)
            ps = psA.tile([P, 512], F32, tag="mm")
            nc.tensor.matmul(out=ps, lhsT=w["wot"][l][:], rhs=cT,
                             start=True, stop=True)
            aoT = ev.tile([P, 512], BF16, tag="aoT")
            nc.scalar.activation(out=aoT, in_=ps, func=AF.Identity,
                                 bias=bias["bo"][l][:])
            for k in range(4):
                j = sl * 4 + k
                pt = psT.tile([P, P], BF16, tag="pt")
                nc.tensor.transpose(out=pt, in_=aoT[:, k * 128:(k + 1) * 128],
                                    identity=c_idb[:])
                nc.vector.tensor_tensor(out=x_f[j], in0=x_f[j], in1=pt,
                                        op=OP.add)

        # ---- FFN ----
        ln_to(xnT)
        for nn in range(NX // 512):
            r1 = []
            for fc in range(4):
                ps = psA.tile([P, 512], F32, tag="mm")
                nc.tensor.matmul(out=ps,
                                 lhsT=w["w1t"][l][:, fc * 128:(fc + 1) * 128],
                                 rhs=xnT[:, nn * 512:(nn + 1) * 512],
                                 start=True, stop=True)
                r1t = ev.tile([P, 512], BF16, tag="r1")
                nc.scalar.activation(out=r1t, in_=ps, func=AF.Relu,
                                     bias=bias["b1_"][l][:, fc:fc + 1])
                r1.append(r1t)
            ps2 = psA.tile([P, 512], F32, tag="mm")
            for fc in range(4):
                nc.tensor.matmul(out=ps2,
                                 lhsT=w["w2t"][l][:, fc * 128:(fc + 1) * 128],
                                 rhs=r1[fc], start=(fc == 0), stop=(fc == 3))
            f2T = ev.tile([P, 512], BF16, tag="aoT")
            nc.scalar.activation(out=f2T, in_=ps2, func=AF.Identity,
                                 bias=bias["b2"][l][:])
            for k in range(4):
                j = nn * 4 + k
                pt = psT.tile([P, P], BF16, tag="pt")
                nc.tensor.transpose(out=pt, in_=f2T[:, k * 128:(k + 1) * 128],
                                    identity=c_idb[:])
                nc.vector.tensor_tensor(out=x_f[j], in0=x_f[j], in1=pt,
                                        op=OP.add)

    # ---- final stage (fp32) ----
    Lgr = persist.tile([P, R], F32)
    nc.vector.memset(Lgr, -1e9)
    Lpair = persist.tile([P, NXT], F32)
    for j in range(NXT):
        tb = ld.tile([P, D], F32, tag="tgtbc", bufs=4)
        for half in range(2):
            eng = nc.sync if half == 0 else nc.gpsimd
            eng.dma_start(out=tb[64 * half:64 * half + 64, :], in_=bass.AP(
                tensor=io["tgt"].tensor, offset=(2 * j + half) * D,
                ap=[[0, 64], [1, D]]))
        scratch = small.tile([P, D], F32, tag="fsc")
        nc.vector.scalar_tensor_tensor(
            out=scratch, in0=x_f[j], scalar=float(SCALE_L), in1=tb,
            op0=OP.mult, op1=OP.mult)
        nc.vector.tensor_reduce(out=Lpair[:, j:j + 1], in_=scratch,
                                axis=mybir.AxisListType.X, op=OP.add)
    # scatter pair columns into per-row columns of Lgr
    for par in range(2):
        lg = Lgr[64 * par:64 * par + 64, :].rearrange("p (j two) -> p j two",
                                                      two=2)
        nc.vector.tensor_copy(
            out=lg[:, :, par:par + 1],
            in_=Lpair[64 * par:64 * par + 64, :].rearrange(
                "p (j o) -> p j o", o=1))
    for par in range(2):
        lp = Lgr[64 * par:64 * par + 64, :].rearrange("p (j two) -> p j two",
                                                      two=2)
        nc.vector.tensor_tensor(
            out=lp[:, :, par:par + 1], in0=lp[:, :, par:par + 1],
            in1=pen_tok[64 * par:64 * par + 64, :].rearrange(
                "p (j o) -> p j o", o=1),
            op=OP.add)
    psL = psX.tile([R, P], F32, tag="ctx")
    nc.tensor.transpose(out=psL, in_=Lgr, identity=c_id[:])
    Erg = persist.tile([R, P], F32)
    den = small.tile([R, 1], F32, tag="den")
    nc.scalar.activation(out=Erg, in_=psL, func=AF.Exp, accum_out=den)
    nc.vector.reciprocal(out=den, in_=den)
    nc.vector.tensor_scalar(out=Erg, in0=Erg, scalar1=den, scalar2=None,
                            op0=OP.mult)
    psW = psX.tile([P, R], F32, tag="ctx")
    nc.tensor.transpose(out=psW, in_=Erg, identity=c_id[0:R, 0:R])
    wT = persist.tile([P, R], F32)
    nc.vector.tensor_copy(out=wT, in_=psW)
    for j in range(NXT):
        psO = psX.tile([2, D], F32, tag="ctx")
        nc.tensor.matmul(out=psO, lhsT=wT[:, 2 * j:2 * j + 2],
                         rhs=x_f[j][:], start=True, stop=True)
        o_sb = ev.tile([2, D], F32, tag="osb")
        nc.vector.tensor_copy(out=o_sb, in_=psO)
        nc.sync.dma_start(out=io["out"][2 * j:2 * j + 2, :], in_=o_sb)


# ---------------------------------------------------------------------------
# host side
# ---------------------------------------------------------------------------

_NC_CACHE = {}


def _get_nc():
    if "nc" not in _NC_CACHE:
        nc = bacc.Bacc("TRN2", target_bir_lowering=False, debug=False,
                       enable_asserts=False)
        _build(nc)
        nc.compile()
        _NC_CACHE["nc"] = nc
    return _NC_CACHE["nc"]


def _consts():
    p = np.arange(128)
    iota_g2 = (p % 64).astype(np.float32)[:, None]
    iota_t = np.tile((np.arange(T) + 1.0).astype(np.float32), (128, 1))
    col = np.arange(NXT)
    rowbase = (256.0 * (2 * col[None, :] + p[:, None] // 64)).astype(np.float32)
    ident = np.eye(128, dtype=np.float32)
    identb = np.eye(128, dtype=ml_dtypes.bfloat16)
    iota_gr = np.tile(np.arange(C, dtype=np.float32), (128, 1))
    return dict(iota_g2=iota_g2, iota_t=iota_t, rowbase=rowbase, ident=ident,
                identb=identb, iota_gr=iota_gr)


def _prep_weights(inp):
    wqkv = np.asarray(inp["wqkv"], np.float32)
    bqkv = np.asarray(inp["bqkv"], np.float32)
    wo = np.asarray(inp["wo"], np.float32)
    bo = np.asarray(inp["bo"], np.float32)
    l1g = np.asarray(inp["ln1_g"], np.float32)
    l1b = np.asarray(inp["ln1_b"], np.float32)
    l2g = np.asarray(inp["ln2_g"], np.float32)
    l2b = np.asarray(inp["ln2_b"], np.float32)
    w1 = np.asarray(inp["w1"], np.float32)
    b1 = np.asarray(inp["b1"], np.float32)
    w2 = np.asarray(inp["w2"], np.float32)
    b2 = np.asarray(inp["b2"], np.float32)
    Wq, Wk, Wv = wqkv[:, :D], wqkv[:, D:2 * D], wqkv[:, 2 * D:]
    bq_, bk_, bv_ = bqkv[:, :D], bqkv[:, D:2 * D], bqkv[:, 2 * D:]
    bf = lambda x: np.ascontiguousarray(x.astype(ml_dtypes.bfloat16))
    f32 = lambda x: np.ascontiguousarray(x.astype(np.float32))
    m = {}
    for l in range(L):
        Wqp = Wq[l] * l1g[l][None, :]
        Wkp = Wk[l] * l1g[l][None, :]
        Wvp = Wv[l] * l1g[l][None, :]
        W1p = w1[l] * l2g[l][None, :]
        bqp = Wq[l] @ l1b[l] + bq_[l]
        bkp = Wk[l] @ l1b[l] + bk_[l]
        bvp = Wv[l] @ l1b[l] + bv_[l]
        b1p = w1[l] @ l2b[l] + b1[l]
        bop = wo[l] @ bvp + bo[l]          # v bias folded through wo
        # v layout on chip: head pairs [v0 | 1 | v1 | 1 | v2 | 1 | v3 | 1]
        m[f"wkt{l}"] = bf(Wkp.T)
        m[f"wvt{l}"] = bf(Wvp.T)
        m[f"wqt{l}"] = bf(Wqp.T)
        m[f"wot{l}"] = bf(wo[l].T)
        m[f"w1t{l}"] = bf(W1p.T)
        w2tl = np.empty((128, F), np.float32)
        for fc in range(4):
            w2tl[:, fc * 128:(fc + 1) * 128] = w2[l][:, fc * 128:(fc + 1) * 128].T
        m[f"w2t{l}"] = bf(w2tl)
        m[f"bk{l}"] = f32(bkp[:, None])
        m[f"bq{l}"] = f32(bqp[:, None])
        m[f"bo{l}"] = f32(bop[:, None])
        m[f"b2{l}"] = f32(b2[l][:, None])
        m[f"b1_{l}"] = f32(b1p.reshape(4, 128).T)
    return m


def kernel(**inputs):
    nc = _get_nc()
    wm = _prep_weights(inputs)
    cm = _consts()
    seq = np.asarray(inputs["sequence_item_emb"], np.float32)
    cat = np.asarray(inputs["sequence_cat_ids"])
    msk = np.asarray(inputs["sequence_mask"])
    tgt = np.asarray(inputs["target_item_emb"], np.float32)
    in_maps = []
    for i in range(NCORES):
        rs = slice(i * R, (i + 1) * R)
        im = dict(wm)
        im.update(cm)
        im["seq"] = np.ascontiguousarray(seq[rs].reshape(NT, D))
        im["seqb"] = im["seq"].astype(ml_dtypes.bfloat16)
        im["catm"] = np.ascontiguousarray(
            np.where(msk[rs], cat[rs], -1).astype(np.int8))
        im["tgt"] = np.ascontiguousarray(tgt[rs])
        in_maps.append(im)
    res = run_bass_kernel_spmd(nc, in_maps, list(range(NCORES)))
    _NC_CACHE["last"] = res
    return np.concatenate([res.results[i]["out"] for i in range(NCORES)], axis=0)


# revision 31
# speedup vs baseline: 1.0311x; 1.0311x over previous
"""Trainium2 Bass kernel for nn_CategoryInterestAttention.

Contract: kernel(**inputs) takes FULL unsharded inputs (as produced by the
problem's setup_inputs) and returns the FULL (512, 128) float32 output.

Strategy (pure data parallel, 8 NeuronCores, 64 batch rows each):
  - Categories are indexed by VALUE g in [0, 64) instead of the reference's
    sorted-unique slot index; softmax over present groups is permutation
    invariant so the final output is identical.
  - match[g,t] = (cat[t]==g)&mask[t]; query of group g = seq embedding of the
    LAST matching position (argmax of (t+1)*match), gathered by indirect DMA.
  - LayerNorm gains/biases are folded into the projection weights host-side;
    the normalized sequence z=(seq-mu)/sigma is computed once on-chip and
    shared by both layers. The v bias is folded into the wo bias (attention
    weights sum to 1 per group).
  - bf16 matmuls with fp32 PSUM accumulation; the final target-attention
    stage runs in fp32 (it dominates the error budget).
  - Attention per (row, t-chunk) in a (t, g) layout:
      scoresT  = kT_chunk.T @ q_blocked + (BIG/s)*match  (two matmuls into
                 one PSUM tile; q head-block-diagonal => 4 heads at once)
      E        = exp(s*scoresT - BIG)     (masking folded into the exp bias;
                                           non-matching entries ~1e-13)
      ctx|den  = E_hh.T @ [v_h0|1|v_h1|1] (ones columns give the softmax
                                           denominators for free)
  - All transposes on the PE (DMA transpose measured ~1.2us each on the sync
    engine); attention/FFN outputs are transposed into PSUM and added to the
    fp32 token-major residual directly from PSUM.
"""

import numpy as np
import ml_dtypes

import concourse.bass as bass
import concourse.bacc as bacc
import concourse.tile as tile
from concourse import mybir
from concourse.bass_utils import run_bass_kernel_spmd

F32 = mybir.dt.float32
BF16 = mybir.dt.bfloat16
I32 = mybir.dt.int32
AF = mybir.ActivationFunctionType
OP = mybir.AluOpType

B, T, D = 512, 256, 128
C, H, L, F = 64, 4, 2, 512
HD = D // H                    # 32
NCORES = 8
R = B // NCORES                # 64 rows per core
NT = R * T                     # 16384 seq tokens per core
NX = R * C                     # 4096 group tokens per core
NTC = NT // 128                # 128 seq chunks
NXT = NX // 128                # 32 x-token tiles
SCALE_S = 1.0 / np.sqrt(np.float32(HD))
SCALE_L = 1.0 / np.sqrt(np.float32(D))
EPS = 1e-5
BIG = 30.0                     # mask offset: exp(-30) ~ 9e-14


def _build(nc):
    seq = nc.dram_tensor("seq", [NT, D], F32, kind="ExternalInput").ap()
    seqb = nc.dram_tensor("seqb", [NT, D], BF16, kind="ExternalInput").ap()
    catm = nc.dram_tensor("catm", [R, T], mybir.dt.int8,
                          kind="ExternalInput").ap()
    tgt = nc.dram_tensor("tgt", [R, D], F32, kind="ExternalInput").ap()
    iota_g2 = nc.dram_tensor("iota_g2", [128, 1], F32, kind="ExternalInput").ap()
    iota_t = nc.dram_tensor("iota_t", [128, T], F32, kind="ExternalInput").ap()
    rowbase = nc.dram_tensor("rowbase", [128, NXT], F32, kind="ExternalInput").ap()
    ident = nc.dram_tensor("ident", [128, 128], F32, kind="ExternalInput").ap()
    identb = nc.dram_tensor("identb", [128, 128], BF16, kind="ExternalInput").ap()
    iota_gr = nc.dram_tensor("iota_gr", [128, C], F32, kind="ExternalInput").ap()
    io = dict(seq=seq, seqb=seqb, catm=catm, tgt=tgt, iota_g2=iota_g2,
              iota_t=iota_t, rowbase=rowbase, ident=ident, identb=identb,
              iota_gr=iota_gr)
    for name, shape, dt_ in [
        ("wkt", [D, D], BF16), ("wvt", [D, D], BF16), ("wqt", [D, D], BF16),
        ("wot", [D, D], BF16), ("w1t", [D, F], BF16), ("w2t", [D, F], BF16),
        ("bk", [D, 1], F32), ("bq", [D, 1], F32), ("bo", [D, 1], F32),
        ("b2", [D, 1], F32), ("b1_", [128, 4], F32),
    ]:
        io[name] = [nc.dram_tensor(f"{name}{l}", shape, dt_,
                                   kind="ExternalInput").ap() for l in range(L)]
    io["out"] = nc.dram_tensor("out", [R, D], F32, kind="ExternalOutput").ap()

    with tile.TileContext(nc) as tc:
        from contextlib import ExitStack
        with ExitStack() as ctx:
            _body(ctx, tc, nc, io)
    return nc


def _body(ctx, tc, nc, io):
    P = 128
    persist = ctx.enter_context(tc.tile_pool(name="persist", bufs=1))
    consts = ctx.enter_context(tc.tile_pool(name="consts", bufs=1))
    ld = ctx.enter_context(tc.tile_pool(name="ld", bufs=4))
    small = ctx.enter_context(tc.tile_pool(name="small", bufs=4))
    ev = ctx.enter_context(tc.tile_pool(name="ev", bufs=4))
    ctokp = ctx.enter_context(tc.tile_pool(name="ctokp", bufs=6))
    epool = ctx.enter_context(tc.tile_pool(name="epool", bufs=4))
    psA = ctx.enter_context(tc.tile_pool(name="psA", bufs=3, space="PSUM"))
    psC = ctx.enter_context(tc.tile_pool(name="psC", bufs=3, space="PSUM"))
    psT = ctx.enter_context(tc.tile_pool(name="psT", bufs=2, space="PSUM"))
    psX = psC

    # ---- constants ----
    c_ig2 = consts.tile([P, 1], F32)
    nc.sync.dma_start(out=c_ig2, in_=io["iota_g2"])
    c_it = consts.tile([P, T], F32)
    nc.sync.dma_start(out=c_it, in_=io["iota_t"])
    c_rb = consts.tile([P, NXT], F32)
    nc.sync.dma_start(out=c_rb, in_=io["rowbase"])
    c_id = consts.tile([P, P], F32)
    nc.sync.dma_start(out=c_id, in_=io["ident"])
    c_idb = consts.tile([P, P], BF16)
    nc.sync.dma_start(out=c_idb, in_=io["identb"])
    c_igr = consts.tile([P, C], F32)
    nc.sync.dma_start(out=c_igr, in_=io["iota_gr"])
    c_eps = consts.tile([P, 1], F32)
    nc.vector.memset(c_eps, EPS)
    w = {}
    for name in ("wkt", "wvt", "wqt", "wot", "w1t", "w2t"):
        w[name] = []
        for l in range(L):
            t_ = consts.tile(list(io[name][l].shape), BF16, tag=f"{name}{l}")
            nc.sync.dma_start(out=t_, in_=io[name][l])
            w[name].append(t_)
    bias = {}
    for name in ("bk", "bq", "bo", "b2", "b1_"):
        bias[name] = []
        for l in range(L):
            t_ = consts.tile(list(io[name][l].shape), F32, tag=f"{name}{l}")
            nc.sync.dma_start(out=t_, in_=io[name][l])
            bias[name].append(t_)

    def pe_transpose_b(dst_sbuf_slice, src_tile, ev_engine=None):
        """bf16 (128,128) transpose via PE + evict into an SBUF slice."""
        pt = psT.tile([P, P], BF16, tag="pt")
        nc.tensor.transpose(out=pt, in_=src_tile, identity=c_idb[:])
        (ev_engine or nc.scalar.copy)(out=dst_sbuf_slice, in_=pt)

    # ---- stage 0: z = LN(seq) token-major -> zT feature-major (bf16) ----
    zT = persist.tile([P, NT], BF16)
    for g4 in range(NT // 512):
        s4 = ld.tile([P, 512], BF16, tag="seqld", bufs=2)
        nc.sync.dma_start(out=s4, in_=bass.AP(
            tensor=io["seqb"].tensor, offset=g4 * 512 * D,
            ap=[[D, 128], [128 * D, 4], [1, D]]))
        mvb = small.tile([P, 4, 2], F32, tag="mvb")
        for j in range(4):
            st = small.tile([P, 6], F32, tag="bnst")
            nc.vector.bn_stats(out=st, in_=s4[:, j * 128:(j + 1) * 128])
            nc.vector.bn_aggr(out=mvb[:, j, :], in_=st)
        nc.scalar.activation(out=mvb[:, :, 1:2], in_=mvb[:, :, 1:2],
                             func=AF.Sqrt, bias=c_eps[:])
        nc.vector.reciprocal(out=mvb[:, :, 1:2], in_=mvb[:, :, 1:2])
        nbias = small.tile([P, 4, 1], F32, tag="nbias")
        nc.vector.scalar_tensor_tensor(out=nbias, in0=mvb[:, :, 0:1],
                                       scalar=-1.0, in1=mvb[:, :, 1:2],
                                       op0=OP.mult, op1=OP.mult)
        z4 = ld.tile([P, 512], BF16, tag="ztok")
        for j in range(4):
            nc.scalar.activation(out=z4[:, j * 128:(j + 1) * 128],
                                 in_=s4[:, j * 128:(j + 1) * 128],
                                 func=AF.Identity, bias=nbias[:, j, :],
                                 scale=mvb[:, j, 1:2])
            pe_transpose_b(zT[:, g4 * 512 + j * 128:g4 * 512 + (j + 1) * 128],
                           z4[:, j * 128:(j + 1) * 128],
                           ev_engine=nc.vector.tensor_copy)

    # ---- stage 1: match (g-layout), qidx, present per row-pair ----
    presentf = persist.tile([P, NXT], F32)
    qposf = persist.tile([P, NXT], F32)
    for rp in range(NXT):
        cat_bc = ld.tile([P, T], mybir.dt.int8, tag="catbc", bufs=2)
        for half in range(2):
            r = 2 * rp + half
            nc.gpsimd.dma_start(out=cat_bc[64 * half:64 * half + 64, :],
                                in_=bass.AP(tensor=io["catm"].tensor,
                                            offset=r * T, ap=[[0, 64], [1, T]]))
        catf_bc = small.tile([P, T], F32, tag="catf", bufs=2)
        nc.vector.tensor_copy(out=catf_bc, in_=cat_bc)
        mg = small.tile([P, T], BF16, tag="mg", bufs=2)
        nc.vector.tensor_scalar(out=mg, in0=catf_bc, scalar1=c_ig2,
                                scalar2=None, op0=OP.is_equal)
        nc.vector.tensor_reduce(out=presentf[:, rp:rp + 1], in_=mg,
                                axis=mybir.AxisListType.X, op=OP.max)
        posm = small.tile([P, T], F32, tag="posm", bufs=2)
        nc.vector.tensor_tensor(out=posm, in0=mg, in1=c_it, op=OP.mult)
        nc.vector.tensor_reduce(out=qposf[:, rp:rp + 1], in_=posm,
                                axis=mybir.AxisListType.X, op=OP.max)
    qidx_i = persist.tile([P, NXT], I32)
    tmpq = small.tile([P, NXT], F32, tag="tmpq")
    nc.vector.tensor_scalar(out=tmpq, in0=qposf, scalar1=-1.0, scalar2=0.0,
                            op0=OP.add, op1=OP.max)
    nc.vector.tensor_tensor(out=tmpq, in0=tmpq, in1=c_rb, op=OP.add)
    nc.vector.tensor_copy(out=qidx_i, in_=tmpq)
    pen_tok = persist.tile([P, NXT], F32)
    nc.vector.tensor_scalar(out=pen_tok, in0=presentf, scalar1=-1.0, scalar2=1e9,
                            op0=OP.add, op1=OP.mult)

    # ---- attention match tiles in (t, g) layout, shared by both layers ----
    cat_tok8 = persist.tile([P, NTC], mybir.dt.int8)
    nc.sync.dma_start(out=cat_tok8, in_=bass.AP(
        tensor=io["catm"].tensor, offset=0, ap=[[1, 128], [T, R], [128, 2]]))
    cat_tok = persist.tile([P, NTC], F32)
    nc.vector.tensor_copy(out=cat_tok, in_=cat_tok8)
    m_tg = [persist.tile([P, 2, 1, C], BF16, tag=f"mtg{r}", name=f"mtg{r}")
            for r in range(R)]
    for r in range(R):
        for c in range(2):
            nc.vector.tensor_scalar(out=m_tg[r][:, c, 0, :], in0=c_igr,
                                    scalar1=cat_tok[:, 2 * r + c:2 * r + c + 1],
                                    scalar2=None, op0=OP.is_equal)

    # ---- x0 gather (token-major fp32 master copy of x) ----
    x_f = [persist.tile([P, D], F32, tag=f"x{j}", name=f"x{j}")
           for j in range(NXT)]
    for j in range(NXT):
        nc.gpsimd.indirect_dma_start(
            out=x_f[j][:], out_offset=None, in_=io["seq"][:],
            in_offset=bass.IndirectOffsetOnAxis(ap=qidx_i[:, j:j + 1], axis=0))

    # ---- per-layer persistent buffers ----
    kT = persist.tile([P, NT], BF16)
    # v[cc]: (128, 4, 33) = [v_h | 1] per head (ones column -> denominators)
    v_sb = [persist.tile([P, H, HD + 1], BF16, tag=f"v{cc}", name=f"v{cc}")
            for cc in range(NTC)]
    for cc in range(NTC):
        nc.vector.memset(v_sb[cc][:, :, HD:HD + 1], 1.0)
    xnT = persist.tile([P, NX], BF16)
    # qb slabs: per 8 rows, q in head-block-diagonal layout (zeros elsewhere
    # memset once; the q evictions always overwrite the same block positions)
    qbs = [persist.tile([P, 8 * H * C], BF16, tag=f"qbs{i}", name=f"qbs{i}")
           for i in range(2)]
    for i in range(2):
        nc.vector.memset(qbs[i], 0.0)

    def ln_to(dst_T):
        for g4 in range(NXT // 4):
            mvb = small.tile([P, 4, 2], F32, tag="mvb")
            for j in range(4):
                st = small.tile([P, 6], F32, tag="bnst")
                nc.vector.bn_stats(out=st, in_=x_f[4 * g4 + j])
                nc.vector.bn_aggr(out=mvb[:, j, :], in_=st)
            nc.scalar.activation(out=mvb[:, :, 1:2], in_=mvb[:, :, 1:2],
                                 func=AF.Sqrt, bias=c_eps[:])
            nc.vector.reciprocal(out=mvb[:, :, 1:2], in_=mvb[:, :, 1:2])
            for j in range(4):
                zx = ld.tile([P, D], BF16, tag="zxtok")
                nc.vector.tensor_scalar(out=zx, in0=x_f[4 * g4 + j],
                                        scalar1=mvb[:, j, 0:1],
                                        scalar2=mvb[:, j, 1:2],
                                        op0=OP.subtract, op1=OP.mult)
                pe_transpose_b(dst_T[:, (4 * g4 + j) * 128:(4 * g4 + j + 1) * 128],
                               zx, ev_engine=nc.vector.tensor_copy)

    for l in range(L):
        # ---- kT = Wk' @ z (feature-major), bias via ACT evict ----
        for nn in range(NT // 512):
            ps = psA.tile([P, 512], F32, tag="mm")
            nc.tensor.matmul(out=ps, lhsT=w["wkt"][l][:],
                             rhs=zT[:, nn * 512:(nn + 1) * 512],
                             start=True, stop=True)
            nc.scalar.activation(out=kT[:, nn * 512:(nn + 1) * 512], in_=ps,
                                 func=AF.Identity, bias=bias["bk"][l][:])
        # ---- v token-major, head-pair layout with ones columns ----
        for cc in range(NTC):
            ps = psA.tile([P, D], F32, tag="mm")
            nc.tensor.matmul(out=ps, lhsT=zT[:, cc * 128:(cc + 1) * 128],
                             rhs=w["wvt"][l][:], start=True, stop=True)
            nc.scalar.copy(
                out=v_sb[cc][:, :, 0:HD],
                in_=ps[:].rearrange("p (h c) -> p h c", h=H))
        # ---- attention; 8 rows (one 512-token slab) at a time ----
        ln_to(xnT)
        for sl in range(NX // 512):
            # q for this slab's 8 rows, evicted into block-diagonal layout
            qsl = qbs[sl % 2]
            ps = psA.tile([P, 512], F32, tag="mm")
            nc.tensor.matmul(out=ps, lhsT=w["wqt"][l][:],
                             rhs=xnT[:, sl * 512:(sl + 1) * 512],
                             start=True, stop=True)
            for h in range(H):
                nc.vector.tensor_scalar(
                    out=qsl[HD * h:HD * (h + 1), :].rearrange(
                        "p (rl q) -> p rl q", q=H * C)[:, :, C * h:C * (h + 1)],
                    in0=ps[HD * h:HD * (h + 1), :].rearrange(
                        "p (rl g) -> p rl g", g=C),
                    scalar1=bias["bq"][l][HD * h:HD * (h + 1), :],
                    scalar2=None, op0=OP.add)
            ctx_tok = []
            for rp2 in range(4):           # row pairs within slab
                psc = psC.tile([P, H, HD + 1], F32, tag="ctx")
                for par in range(2):
                    r = 8 * sl + 2 * rp2 + par
                    rl = 2 * rp2 + par
                    qb = qsl[:, rl * H * C:(rl + 1) * H * C]
                    ps = psA.tile([P, 2 * H * C], F32, tag="mm")
                    for c in range(2):
                        cc = 2 * r + c
                        nc.tensor.matmul(out=ps[:, 256 * c:256 * (c + 1)],
                                         lhsT=kT[:, cc * 128:(cc + 1) * 128],
                                         rhs=qb, start=(c == 0), stop=(c == 1))
                    et = epool.tile([P, 2 * H * C], BF16, tag="et")
                    nc.scalar.activation(out=et, in_=ps, func=AF.Exp,
                                         scale=float(SCALE_S))
                    E2 = epool.tile([P, 2 * H * C], BF16, tag="E")
                    nc.vector.tensor_tensor(
                        out=E2[:].rearrange("p (c h g) -> p c h g", c=2, h=H),
                        in0=et[:].rearrange("p (c h g) -> p c h g", c=2, h=H),
                        in1=m_tg[r].to_broadcast([P, 2, H, C]), op=OP.mult)
                    off = 64 * par
                    for h in range(H):
                        for c in range(2):
                            nc.tensor.matmul(
                                out=psc[off:off + 64, h, :],
                                lhsT=E2[:, 256 * c + C * h:256 * c + C * (h + 1)],
                                rhs=v_sb[2 * r + c][:, h, :],
                                start=(h == 0 and c == 0),
                                stop=(h == H - 1 and c == 1))
                rd = small.tile([P, H, 1], F32, tag="rd")
                nc.vector.tensor_scalar(out=rd, in0=psc[:, :, HD:HD + 1],
                                        scalar1=1e-30, scalar2=None, op0=OP.add)
                nc.vector.reciprocal(out=rd, in_=rd)
                ct = ctokp.tile([P, D], BF16, tag="ctok")
                nc.vector.scalar_tensor_tensor(
                    out=ct[:].rearrange("p (h c) -> p h c", h=H),
                    in0=psc[:, :, 0:HD], scalar=1.0,
                    in1=rd.to_broadcast([P, H, HD]),
                    op0=OP.mult, op1=OP.mult)
                ctx_tok.append(ct)
            cT = ev.tile([P, 512], BF16, tag="cT")
            for k in range(4):
                pe_transpose_b(cT[:, k * 128:(k + 1) * 128], ctx_tok[k],
                               ev_engine=nc.vector.tensor_copy)
            ps = psA.tile([P, 512], F32, tag="mm")
            nc.tensor.matmul(out=ps, lhsT=w["wot"][l][:], rhs=cT,
                             start=True, stop=True)
            aoT = ev.tile([P, 512], BF16, tag="aoT")
            nc.scalar.activation(out=aoT, in_=ps, func=AF.Identity,
                                 bias=bias["bo"][l][:])
            for k in range(4):
                j = sl * 4 + k
                pt = psT.tile([P, P], BF16, tag="pt")
                nc.tensor.transpose(out=pt, in_=aoT[:, k * 128:(k + 1) * 128],
                                    identity=c_idb[:])
                nc.vector.tensor_tensor(out=x_f[j], in0=x_f[j], in1=pt,
                                        op=OP.add)

        # ---- FFN ----
        ln_to(xnT)
        for nn in range(NX // 512):
            r1 = []
            for fc in range(4):
                ps = psA.tile([P, 512], F32, tag="mm")
                nc.tensor.matmul(out=ps,
                                 lhsT=w["w1t"][l][:, fc * 128:(fc + 1) * 128],
                                 rhs=xnT[:, nn * 512:(nn + 1) * 512],
                                 start=True, stop=True)
                r1t = ev.tile([P, 512], BF16, tag="r1")
                nc.scalar.activation(out=r1t, in_=ps, func=AF.Relu,
                                     bias=bias["b1_"][l][:, fc:fc + 1])
                r1.append(r1t)
            ps2 = psA.tile([P, 512], F32, tag="mm")
            for fc in range(4):
                nc.tensor.matmul(out=ps2,
                                 lhsT=w["w2t"][l][:, fc * 128:(fc + 1) * 128],
                                 rhs=r1[fc], start=(fc == 0), stop=(fc == 3))
            f2T = ev.tile([P, 512], BF16, tag="aoT")
            nc.scalar.activation(out=f2T, in_=ps2, func=AF.Identity,
                                 bias=bias["b2"][l][:])
            for k in range(4):
                j = nn * 4 + k
                pt = psT.tile([P, P], BF16, tag="pt")
                nc.tensor.transpose(out=pt, in_=f2T[:, k * 128:(k + 1) * 128],
                                    identity=c_idb[:])
                nc.vector.tensor_tensor(out=x_f[j], in0=x_f[j], in1=pt,
                                        op=OP.add)

    # ---- final stage (fp32) ----
    Lgr = persist.tile([P, R], F32)
    nc.vector.memset(Lgr, -1e9)
    Lpair = persist.tile([P, NXT], F32)
    for j in range(NXT):
        tb = ld.tile([P, D], F32, tag="tgtbc", bufs=4)
        for half in range(2):
            eng = nc.sync if half == 0 else nc.gpsimd
            eng.dma_start(out=tb[64 * half:64 * half + 64, :], in_=bass.AP(
                tensor=io["tgt"].tensor, offset=(2 * j + half) * D,
                ap=[[0, 64], [1, D]]))
        scratch = small.tile([P, D], F32, tag="fsc")
        nc.vector.scalar_tensor_tensor(
            out=scratch, in0=x_f[j], scalar=float(SCALE_L), in1=tb,
            op0=OP.mult, op1=OP.mult)
        nc.vector.tensor_reduce(out=Lpair[:, j:j + 1], in_=scratch,
                                axis=mybir.AxisListType.X, op=OP.add)
    # scatter pair columns into per-row columns of Lgr
    for par in range(2):
        lg = Lgr[64 * par:64 * par + 64, :].rearrange("p (j two) -> p j two",
                                                      two=2)
        nc.vector.tensor_copy(
            out=lg[:, :, par:par + 1],
            in_=Lpair[64 * par:64 * par + 64, :].rearrange(
                "p (j o) -> p j o", o=1))
    for par in range(2):
        lp = Lgr[64 * par:64 * par + 64, :].rearrange("p (j two) -> p j two",
                                                      two=2)
        nc.vector.tensor_tensor(
            out=lp[:, :, par:par + 1], in0=lp[:, :, par:par + 1],
            in1=pen_tok[64 * par:64 * par + 64, :].rearrange(
                "p (j o) -> p j o", o=1),
            op=OP.add)
    psL = psX.tile([R, P], F32, tag="ctx")
    nc.tensor.transpose(out=psL, in_=Lgr, identity=c_id[:])
    Erg = persist.tile([R, P], F32)
    den = small.tile([R, 1], F32, tag="den")
    nc.scalar.activation(out=Erg, in_=psL, func=AF.Exp, accum_out=den)
    nc.vector.reciprocal(out=den, in_=den)
    nc.vector.tensor_scalar(out=Erg, in0=Erg, scalar1=den, scalar2=None,
                            op0=OP.mult)
    psW = psX.tile([P, R], F32, tag="ctx")
    nc.tensor.transpose(out=psW, in_=Erg, identity=c_id[0:R, 0:R])
    wT = persist.tile([P, R], F32)
    nc.vector.tensor_copy(out=wT, in_=psW)
    for j in range(NXT):
        psO = psX.tile([2, D], F32, tag="ctx")
        nc.tensor.matmul(out=psO, lhsT=wT[:, 2 * j:2 * j + 2],
                         rhs=x_f[j][:], start=True, stop=True)
        o_sb = ev.tile([2, D], F32, tag="osb")
        nc.vector.tensor_copy(out=o_sb, in_=psO)
        nc.sync.dma_start(out=io["out"][2 * j:2 * j + 2, :], in_=o_sb)


# ---------------------------------------------------------------------------
# host side
# ---------------------------------------------------------------------------

_NC_CACHE = {}


def _get_nc():
    if "nc" not in _NC_CACHE:
        nc = bacc.Bacc("TRN2", target_bir_lowering=False, debug=False,
                       enable_asserts=False)
        _build(nc)
        nc.compile()
        _NC_CACHE["nc"] = nc
    return _NC_CACHE["nc"]


def _consts():
    p = np.arange(128)
    iota_g2 = (p % 64).astype(np.float32)[:, None]
    iota_t = np.tile((np.arange(T) + 1.0).astype(np.float32), (128, 1))
    col = np.arange(NXT)
    rowbase = (256.0 * (2 * col[None, :] + p[:, None] // 64)).astype(np.float32)
    ident = np.eye(128, dtype=np.float32)
    identb = np.eye(128, dtype=ml_dtypes.bfloat16)
    iota_gr = np.tile(np.arange(C, dtype=np.float32), (128, 1))
    return dict(iota_g2=iota_g2, iota_t=iota_t, rowbase=rowbase, ident=ident,
                identb=identb, iota_gr=iota_gr)


def _prep_weights(inp):
    wqkv = np.asarray(inp["wqkv"], np.float32)
    bqkv = np.asarray(inp["bqkv"], np.float32)
    wo = np.asarray(inp["wo"], np.float32)
    bo = np.asarray(inp["bo"], np.float32)
    l1g = np.asarray(inp["ln1_g"], np.float32)
    l1b = np.asarray(inp["ln1_b"], np.float32)
    l2g = np.asarray(inp["ln2_g"], np.float32)
    l2b = np.asarray(inp["ln2_b"], np.float32)
    w1 = np.asarray(inp["w1"], np.float32)
    b1 = np.asarray(inp["b1"], np.float32)
    w2 = np.asarray(inp["w2"], np.float32)
    b2 = np.asarray(inp["b2"], np.float32)
    Wq, Wk, Wv = wqkv[:, :D], wqkv[:, D:2 * D], wqkv[:, 2 * D:]
    bq_, bk_, bv_ = bqkv[:, :D], bqkv[:, D:2 * D], bqkv[:, 2 * D:]
    bf = lambda x: np.ascontiguousarray(x.astype(ml_dtypes.bfloat16))
    f32 = lambda x: np.ascontiguousarray(x.astype(np.float32))
    m = {}
    for l in range(L):
        Wqp = Wq[l] * l1g[l][None, :]
        Wkp = Wk[l] * l1g[l][None, :]
        Wvp = Wv[l] * l1g[l][None, :]
        W1p = w1[l] * l2g[l][None, :]
        bqp = Wq[l] @ l1b[l] + bq_[l]
        bkp = Wk[l] @ l1b[l] + bk_[l]
        bvp = Wv[l] @ l1b[l] + bv_[l]
        b1p = w1[l] @ l2b[l] + b1[l]
        bop = wo[l] @ bvp + bo[l]          # v bias folded through wo
        # v layout on chip: head pairs [v0 | 1 | v1 | 1 | v2 | 1 | v3 | 1]
        m[f"wkt{l}"] = bf(Wkp.T)
        m[f"wvt{l}"] = bf(Wvp.T)
        m[f"wqt{l}"] = bf(Wqp.T)
        m[f"wot{l}"] = bf(wo[l].T)
        m[f"w1t{l}"] = bf(W1p.T)
        w2tl = np.empty((128, F), np.float32)
        for fc in range(4):
            w2tl[:, fc * 128:(fc + 1) * 128] = w2[l][:, fc * 128:(fc + 1) * 128].T
        m[f"w2t{l}"] = bf(w2tl)
        m[f"bk{l}"] = f32(bkp[:, None])
        m[f"bq{l}"] = f32(bqp[:, None])
        m[f"bo{l}"] = f32(bop[:, None])
        m[f"b2{l}"] = f32(b2[l][:, None])
        m[f"b1_{l}"] = f32(b1p.reshape(4, 128).T)
    return m


def kernel(**inputs):
    nc = _get_nc()
    wm = _prep_weights(inputs)
    cm = _consts()
    seq = np.asarray(inputs["sequence_item_emb"], np.float32)
    cat = np.asarray(inputs["sequence_cat_ids"])
    msk = np.asarray(inputs["sequence_mask"])
    tgt = np.asarray(inputs["target_item_emb"], np.float32)
    in_maps = []
    for i in range(NCORES):
        rs = slice(i * R, (i + 1) * R)
        im = dict(wm)
        im.update(cm)
        im["seq"] = np.ascontiguousarray(seq[rs].reshape(NT, D))
        im["seqb"] = im["seq"].astype(ml_dtypes.bfloat16)
        im["catm"] = np.ascontiguousarray(
            np.where(msk[rs], cat[rs], -1).astype(np.int8))
        im["tgt"] = np.ascontiguousarray(tgt[rs])
        in_maps.append(im)
    res = run_bass_kernel_spmd(nc, in_maps, list(range(NCORES)))
    _NC_CACHE["last"] = res
    return np.concatenate([res.results[i]["out"] for i in range(NCORES)], axis=0)


# revision 32
# speedup vs baseline: 1.0812x; 1.0486x over previous
"""Trainium2 Bass kernel for nn_CategoryInterestAttention.

Contract: kernel(**inputs) takes FULL unsharded inputs (as produced by the
problem's setup_inputs) and returns the FULL (512, 128) float32 output.

Strategy (pure data parallel, 8 NeuronCores, 64 batch rows each):
  - Categories are indexed by VALUE g in [0, 64) instead of the reference's
    sorted-unique slot index; softmax over present groups is permutation
    invariant so the final output is identical.
  - match[g,t] = (cat[t]==g)&mask[t]; query of group g = seq embedding of the
    LAST matching position (argmax of (t+1)*match), gathered by indirect DMA.
  - LayerNorm gains/biases are folded into the projection weights host-side;
    the normalized sequence z=(seq-mu)/sigma is computed once on-chip and
    shared by both layers. The v bias is folded into the wo bias (attention
    weights sum to 1 per group).
  - bf16 matmuls with fp32 PSUM accumulation; the final target-attention
    stage runs in fp32 (it dominates the error budget).
  - Attention per (row, t-chunk) in a (t, g) layout:
      scoresT  = kT_chunk.T @ q_blocked + (BIG/s)*match  (two matmuls into
                 one PSUM tile; q head-block-diagonal => 4 heads at once)
      E        = exp(s*scoresT - BIG)     (masking folded into the exp bias;
                                           non-matching entries ~1e-13)
      ctx|den  = E_hh.T @ [v_h0|1|v_h1|1] (ones columns give the softmax
                                           denominators for free)
  - All transposes on the PE (DMA transpose measured ~1.2us each on the sync
    engine); attention/FFN outputs are transposed into PSUM and added to the
    fp32 token-major residual directly from PSUM.
"""

import numpy as np
import ml_dtypes

import concourse.bass as bass
import concourse.bacc as bacc
import concourse.tile as tile
from concourse import mybir
from concourse.bass_utils import run_bass_kernel_spmd

F32 = mybir.dt.float32
BF16 = mybir.dt.bfloat16
I32 = mybir.dt.int32
AF = mybir.ActivationFunctionType
OP = mybir.AluOpType

B, T, D = 512, 256, 128
C, H, L, F = 64, 4, 2, 512
HD = D // H                    # 32
NCORES = 8
R = B // NCORES                # 64 rows per core
NT = R * T                     # 16384 seq tokens per core
NX = R * C                     # 4096 group tokens per core
NTC = NT // 128                # 128 seq chunks
NXT = NX // 128                # 32 x-token tiles
SCALE_S = 1.0 / np.sqrt(np.float32(HD))
SCALE_L = 1.0 / np.sqrt(np.float32(D))
EPS = 1e-5
BIG = 30.0                     # mask offset: exp(-30) ~ 9e-14


def _build(nc):
    seq = nc.dram_tensor("seq", [NT, D], F32, kind="ExternalInput").ap()
    seqb = nc.dram_tensor("seqb", [NT, D], BF16, kind="ExternalInput").ap()
    catm = nc.dram_tensor("catm", [R, T], mybir.dt.int8,
                          kind="ExternalInput").ap()
    tgt = nc.dram_tensor("tgt", [R, D], F32, kind="ExternalInput").ap()
    iota_g2 = nc.dram_tensor("iota_g2", [128, 1], F32, kind="ExternalInput").ap()
    iota_t = nc.dram_tensor("iota_t", [128, T], F32, kind="ExternalInput").ap()
    rowbase = nc.dram_tensor("rowbase", [128, NXT], F32, kind="ExternalInput").ap()
    ident = nc.dram_tensor("ident", [128, 128], F32, kind="ExternalInput").ap()
    identb = nc.dram_tensor("identb", [128, 128], BF16, kind="ExternalInput").ap()
    iota_gr = nc.dram_tensor("iota_gr", [128, C], F32, kind="ExternalInput").ap()
    io = dict(seq=seq, seqb=seqb, catm=catm, tgt=tgt, iota_g2=iota_g2,
              iota_t=iota_t, rowbase=rowbase, ident=ident, identb=identb,
              iota_gr=iota_gr)
    for name, shape, dt_ in [
        ("wkt", [D, D], BF16), ("wvt", [D, D], BF16), ("wqt", [D, D], BF16),
        ("wot", [D, D], BF16), ("w1t", [D, F], BF16), ("w2t", [D, F], BF16),
        ("bk", [D, 1], F32), ("bq", [D, 1], F32), ("bo", [D, 1], F32),
        ("b2", [D, 1], F32), ("b1_", [128, 4], F32),
    ]:
        io[name] = [nc.dram_tensor(f"{name}{l}", shape, dt_,
                                   kind="ExternalInput").ap() for l in range(L)]
    io["out"] = nc.dram_tensor("out", [R, D], F32, kind="ExternalOutput").ap()

    with tile.TileContext(nc) as tc:
        from contextlib import ExitStack
        with ExitStack() as ctx:
            _body(ctx, tc, nc, io)
    return nc


def _body(ctx, tc, nc, io):
    P = 128
    persist = ctx.enter_context(tc.tile_pool(name="persist", bufs=1))
    consts = ctx.enter_context(tc.tile_pool(name="consts", bufs=1))
    ld = ctx.enter_context(tc.tile_pool(name="ld", bufs=4))
    small = ctx.enter_context(tc.tile_pool(name="small", bufs=4))
    ev = ctx.enter_context(tc.tile_pool(name="ev", bufs=4))
    ctokp = ctx.enter_context(tc.tile_pool(name="ctokp", bufs=6))
    epool = ctx.enter_context(tc.tile_pool(name="epool", bufs=4))
    psA = ctx.enter_context(tc.tile_pool(name="psA", bufs=3, space="PSUM"))
    psC = ctx.enter_context(tc.tile_pool(name="psC", bufs=3, space="PSUM"))
    psT = ctx.enter_context(tc.tile_pool(name="psT", bufs=2, space="PSUM"))
    psX = psC

    # ---- constants ----
    c_ig2 = consts.tile([P, 1], F32)
    nc.sync.dma_start(out=c_ig2, in_=io["iota_g2"])
    c_it = consts.tile([P, T], F32)
    nc.sync.dma_start(out=c_it, in_=io["iota_t"])
    c_rb = consts.tile([P, NXT], F32)
    nc.sync.dma_start(out=c_rb, in_=io["rowbase"])
    c_id = consts.tile([P, P], F32)
    nc.sync.dma_start(out=c_id, in_=io["ident"])
    c_idb = consts.tile([P, P], BF16)
    nc.sync.dma_start(out=c_idb, in_=io["identb"])
    c_igr = consts.tile([P, C], F32)
    nc.sync.dma_start(out=c_igr, in_=io["iota_gr"])
    c_eps = consts.tile([P, 1], F32)
    nc.vector.memset(c_eps, EPS)
    w = {}
    for name in ("wkt", "wvt", "wqt", "wot", "w1t", "w2t"):
        w[name] = []
        for l in range(L):
            t_ = consts.tile(list(io[name][l].shape), BF16, tag=f"{name}{l}")
            nc.sync.dma_start(out=t_, in_=io[name][l])
            w[name].append(t_)
    bias = {}
    for name in ("bk", "bq", "bo", "b2", "b1_"):
        bias[name] = []
        for l in range(L):
            t_ = consts.tile(list(io[name][l].shape), F32, tag=f"{name}{l}")
            nc.sync.dma_start(out=t_, in_=io[name][l])
            bias[name].append(t_)

    def pe_transpose_b(dst_sbuf_slice, src_tile, ev_engine=None):
        """bf16 (128,128) transpose via PE + evict into an SBUF slice."""
        pt = psT.tile([P, P], BF16, tag="pt")
        nc.tensor.transpose(out=pt, in_=src_tile, identity=c_idb[:])
        (ev_engine or nc.scalar.copy)(out=dst_sbuf_slice, in_=pt)

    # ---- stage 0: z = LN(seq) token-major -> zT feature-major (bf16) ----
    zT = persist.tile([P, NT], BF16)
    for g4 in range(NT // 512):
        s4 = ld.tile([P, 512], BF16, tag="seqld", bufs=2)
        nc.sync.dma_start(out=s4, in_=bass.AP(
            tensor=io["seqb"].tensor, offset=g4 * 512 * D,
            ap=[[D, 128], [128 * D, 4], [1, D]]))
        mvb = small.tile([P, 4, 2], F32, tag="mvb")
        for j in range(4):
            st = small.tile([P, 6], F32, tag="bnst")
            nc.vector.bn_stats(out=st, in_=s4[:, j * 128:(j + 1) * 128])
            nc.vector.bn_aggr(out=mvb[:, j, :], in_=st)
        nc.scalar.activation(out=mvb[:, :, 1:2], in_=mvb[:, :, 1:2],
                             func=AF.Sqrt, bias=c_eps[:])
        nc.vector.reciprocal(out=mvb[:, :, 1:2], in_=mvb[:, :, 1:2])
        z4 = ld.tile([P, 512], BF16, tag="ztok")
        for j in range(4):
            nc.vector.tensor_scalar(
                out=z4[:, j * 128:(j + 1) * 128],
                in0=s4[:, j * 128:(j + 1) * 128], scalar1=mvb[:, j, 0:1],
                scalar2=mvb[:, j, 1:2], op0=OP.subtract, op1=OP.mult)
            pe_transpose_b(zT[:, g4 * 512 + j * 128:g4 * 512 + (j + 1) * 128],
                           z4[:, j * 128:(j + 1) * 128])

    # ---- stage 1: match (g-layout), qidx, present per row-pair ----
    presentf = persist.tile([P, NXT], F32)
    qposf = persist.tile([P, NXT], F32)
    for rp in range(NXT):
        cat_bc = ld.tile([P, T], mybir.dt.int8, tag="catbc", bufs=2)
        for half in range(2):
            r = 2 * rp + half
            nc.gpsimd.dma_start(out=cat_bc[64 * half:64 * half + 64, :],
                                in_=bass.AP(tensor=io["catm"].tensor,
                                            offset=r * T, ap=[[0, 64], [1, T]]))
        catf_bc = small.tile([P, T], F32, tag="catf", bufs=2)
        nc.vector.tensor_copy(out=catf_bc, in_=cat_bc)
        mg = small.tile([P, T], BF16, tag="mg", bufs=2)
        nc.vector.tensor_scalar(out=mg, in0=catf_bc, scalar1=c_ig2,
                                scalar2=None, op0=OP.is_equal)
        nc.vector.tensor_reduce(out=presentf[:, rp:rp + 1], in_=mg,
                                axis=mybir.AxisListType.X, op=OP.max)
        posm = small.tile([P, T], F32, tag="posm", bufs=2)
        nc.vector.tensor_tensor(out=posm, in0=mg, in1=c_it, op=OP.mult)
        nc.vector.tensor_reduce(out=qposf[:, rp:rp + 1], in_=posm,
                                axis=mybir.AxisListType.X, op=OP.max)
    qidx_i = persist.tile([P, NXT], I32)
    tmpq = small.tile([P, NXT], F32, tag="tmpq")
    nc.vector.tensor_scalar(out=tmpq, in0=qposf, scalar1=-1.0, scalar2=0.0,
                            op0=OP.add, op1=OP.max)
    nc.vector.tensor_tensor(out=tmpq, in0=tmpq, in1=c_rb, op=OP.add)
    nc.vector.tensor_copy(out=qidx_i, in_=tmpq)
    pen_tok = persist.tile([P, NXT], F32)
    nc.vector.tensor_scalar(out=pen_tok, in0=presentf, scalar1=-1.0, scalar2=1e9,
                            op0=OP.add, op1=OP.mult)

    # ---- attention match tiles in (t, g) layout, shared by both layers ----
    cat_tok8 = persist.tile([P, NTC], mybir.dt.int8)
    nc.sync.dma_start(out=cat_tok8, in_=bass.AP(
        tensor=io["catm"].tensor, offset=0, ap=[[1, 128], [T, R], [128, 2]]))
    cat_tok = persist.tile([P, NTC], F32)
    nc.vector.tensor_copy(out=cat_tok, in_=cat_tok8)
    m_tg = [persist.tile([P, 2, 1, C], BF16, tag=f"mtg{r}", name=f"mtg{r}")
            for r in range(R)]
    for r in range(R):
        for c in range(2):
            nc.vector.tensor_scalar(out=m_tg[r][:, c, 0, :], in0=c_igr,
                                    scalar1=cat_tok[:, 2 * r + c:2 * r + c + 1],
                                    scalar2=None, op0=OP.is_equal)

    # ---- x0 gather (token-major fp32 master copy of x) ----
    x_f = [persist.tile([P, D], F32, tag=f"x{j}", name=f"x{j}")
           for j in range(NXT)]
    for j in range(NXT):
        nc.gpsimd.indirect_dma_start(
            out=x_f[j][:], out_offset=None, in_=io["seq"][:],
            in_offset=bass.IndirectOffsetOnAxis(ap=qidx_i[:, j:j + 1], axis=0))

    # ---- per-layer persistent buffers ----
    kT = persist.tile([P, NT], BF16)
    # v[cc]: (128, 4, 33) = [v_h | 1] per head (ones column -> denominators)
    v_sb = [persist.tile([P, H, HD + 1], BF16, tag=f"v{cc}", name=f"v{cc}")
            for cc in range(NTC)]
    for cc in range(NTC):
        nc.vector.memset(v_sb[cc][:, :, HD:HD + 1], 1.0)
    xnT = persist.tile([P, NX], BF16)
    # qb slabs: per 8 rows, q in head-block-diagonal layout (zeros elsewhere
    # memset once; the q evictions always overwrite the same block positions)
    qbs = [persist.tile([P, 8 * H * C], BF16, tag=f"qbs{i}", name=f"qbs{i}")
           for i in range(2)]
    for i in range(2):
        nc.vector.memset(qbs[i], 0.0)

    def ln_to(dst_T):
        for g4 in range(NXT // 4):
            mvb = small.tile([P, 4, 2], F32, tag="mvb")
            for j in range(4):
                st = small.tile([P, 6], F32, tag="bnst")
                nc.vector.bn_stats(out=st, in_=x_f[4 * g4 + j])
                nc.vector.bn_aggr(out=mvb[:, j, :], in_=st)
            nc.scalar.activation(out=mvb[:, :, 1:2], in_=mvb[:, :, 1:2],
                                 func=AF.Sqrt, bias=c_eps[:])
            nc.vector.reciprocal(out=mvb[:, :, 1:2], in_=mvb[:, :, 1:2])
            for j in range(4):
                zx = ld.tile([P, D], BF16, tag="zxtok")
                nc.vector.tensor_scalar(out=zx, in0=x_f[4 * g4 + j],
                                        scalar1=mvb[:, j, 0:1],
                                        scalar2=mvb[:, j, 1:2],
                                        op0=OP.subtract, op1=OP.mult)
                pe_transpose_b(dst_T[:, (4 * g4 + j) * 128:(4 * g4 + j + 1) * 128],
                               zx, ev_engine=nc.vector.tensor_copy)

    for l in range(L):
        # ---- kT = Wk' @ z (feature-major), bias via ACT evict ----
        for nn in range(NT // 512):
            ps = psA.tile([P, 512], F32, tag="mm")
            nc.tensor.matmul(out=ps, lhsT=w["wkt"][l][:],
                             rhs=zT[:, nn * 512:(nn + 1) * 512],
                             start=True, stop=True)
            nc.scalar.activation(out=kT[:, nn * 512:(nn + 1) * 512], in_=ps,
                                 func=AF.Identity, bias=bias["bk"][l][:])
        # ---- v token-major, head-pair layout with ones columns ----
        for cc in range(NTC):
            ps = psA.tile([P, D], F32, tag="mm")
            nc.tensor.matmul(out=ps, lhsT=zT[:, cc * 128:(cc + 1) * 128],
                             rhs=w["wvt"][l][:], start=True, stop=True)
            nc.scalar.copy(
                out=v_sb[cc][:, :, 0:HD],
                in_=ps[:].rearrange("p (h c) -> p h c", h=H))
        # ---- attention; 8 rows (one 512-token slab) at a time ----
        ln_to(xnT)
        for sl in range(NX // 512):
            # q for this slab's 8 rows, evicted into block-diagonal layout
            qsl = qbs[sl % 2]
            ps = psA.tile([P, 512], F32, tag="mm")
            nc.tensor.matmul(out=ps, lhsT=w["wqt"][l][:],
                             rhs=xnT[:, sl * 512:(sl + 1) * 512],
                             start=True, stop=True)
            for h in range(H):
                nc.vector.tensor_scalar(
                    out=qsl[HD * h:HD * (h + 1), :].rearrange(
                        "p (rl q) -> p rl q", q=H * C)[:, :, C * h:C * (h + 1)],
                    in0=ps[HD * h:HD * (h + 1), :].rearrange(
                        "p (rl g) -> p rl g", g=C),
                    scalar1=bias["bq"][l][HD * h:HD * (h + 1), :],
                    scalar2=None, op0=OP.add)
            ctx_tok = []
            for rp2 in range(4):           # row pairs within slab
                psc = psC.tile([P, H, HD + 1], F32, tag="ctx")
                for par in range(2):
                    r = 8 * sl + 2 * rp2 + par
                    rl = 2 * rp2 + par
                    qb = qsl[:, rl * H * C:(rl + 1) * H * C]
                    ps = psA.tile([P, 2 * H * C], F32, tag="mm")
                    for c in range(2):
                        cc = 2 * r + c
                        nc.tensor.matmul(out=ps[:, 256 * c:256 * (c + 1)],
                                         lhsT=kT[:, cc * 128:(cc + 1) * 128],
                                         rhs=qb, start=(c == 0), stop=(c == 1))
                    et = epool.tile([P, 2 * H * C], BF16, tag="et")
                    nc.scalar.activation(out=et, in_=ps, func=AF.Exp,
                                         scale=float(SCALE_S))
                    E2 = epool.tile([P, 2 * H * C], BF16, tag="E")
                    nc.vector.tensor_tensor(
                        out=E2[:].rearrange("p (c h g) -> p c h g", c=2, h=H),
                        in0=et[:].rearrange("p (c h g) -> p c h g", c=2, h=H),
                        in1=m_tg[r].to_broadcast([P, 2, H, C]), op=OP.mult)
                    off = 64 * par
                    for h in range(H):
                        for c in range(2):
                            nc.tensor.matmul(
                                out=psc[off:off + 64, h, :],
                                lhsT=E2[:, 256 * c + C * h:256 * c + C * (h + 1)],
                                rhs=v_sb[2 * r + c][:, h, :],
                                start=(h == 0 and c == 0),
                                stop=(h == H - 1 and c == 1))
                rd = small.tile([P, H, 1], F32, tag="rd")
                nc.vector.tensor_scalar(out=rd, in0=psc[:, :, HD:HD + 1],
                                        scalar1=1e-30, scalar2=None, op0=OP.add)
                nc.vector.reciprocal(out=rd, in_=rd)
                ct = ctokp.tile([P, D], BF16, tag="ctok")
                nc.vector.scalar_tensor_tensor(
                    out=ct[:].rearrange("p (h c) -> p h c", h=H),
                    in0=psc[:, :, 0:HD], scalar=1.0,
                    in1=rd.to_broadcast([P, H, HD]),
                    op0=OP.mult, op1=OP.mult)
                ctx_tok.append(ct)
            cT = ev.tile([P, 512], BF16, tag="cT")
            for k in range(4):
                pe_transpose_b(cT[:, k * 128:(k + 1) * 128], ctx_tok[k],
                               ev_engine=nc.vector.tensor_copy)
            ps = psA.tile([P, 512], F32, tag="mm")
            nc.tensor.matmul(out=ps, lhsT=w["wot"][l][:], rhs=cT,
                             start=True, stop=True)
            aoT = ev.tile([P, 512], BF16, tag="aoT")
            nc.scalar.activation(out=aoT, in_=ps, func=AF.Identity,
                                 bias=bias["bo"][l][:])
            for k in range(4):
                j = sl * 4 + k
                pt = psT.tile([P, P], BF16, tag="pt")
                nc.tensor.transpose(out=pt, in_=aoT[:, k * 128:(k + 1) * 128],
                                    identity=c_idb[:])
                nc.vector.tensor_tensor(out=x_f[j], in0=x_f[j], in1=pt,
                                        op=OP.add)

        # ---- FFN ----
        ln_to(xnT)
        for nn in range(NX // 512):
            r1 = []
            for fc in range(4):
                ps = psA.tile([P, 512], F32, tag="mm")
                nc.tensor.matmul(out=ps,
                                 lhsT=w["w1t"][l][:, fc * 128:(fc + 1) * 128],
                                 rhs=xnT[:, nn * 512:(nn + 1) * 512],
                                 start=True, stop=True)
                r1t = ev.tile([P, 512], BF16, tag="r1")
                nc.scalar.activation(out=r1t, in_=ps, func=AF.Relu,
                                     bias=bias["b1_"][l][:, fc:fc + 1])
                r1.append(r1t)
            ps2 = psA.tile([P, 512], F32, tag="mm")
            for fc in range(4):
                nc.tensor.matmul(out=ps2,
                                 lhsT=w["w2t"][l][:, fc * 128:(fc + 1) * 128],
                                 rhs=r1[fc], start=(fc == 0), stop=(fc == 3))
            f2T = ev.tile([P, 512], BF16, tag="aoT")
            nc.scalar.activation(out=f2T, in_=ps2, func=AF.Identity,
                                 bias=bias["b2"][l][:])
            for k in range(4):
                j = nn * 4 + k
                pt = psT.tile([P, P], BF16, tag="pt")
                nc.tensor.transpose(out=pt, in_=f2T[:, k * 128:(k + 1) * 128],
                                    identity=c_idb[:])
                nc.vector.tensor_tensor(out=x_f[j], in0=x_f[j], in1=pt,
                                        op=OP.add)

    # ---- final stage (fp32) ----
    Lgr = persist.tile([P, R], F32)
    nc.vector.memset(Lgr, -1e9)
    Lpair = persist.tile([P, NXT], F32)
    for j in range(NXT):
        tb = ld.tile([P, D], F32, tag="tgtbc", bufs=4)
        for half in range(2):
            eng = nc.sync if half == 0 else nc.gpsimd
            eng.dma_start(out=tb[64 * half:64 * half + 64, :], in_=bass.AP(
                tensor=io["tgt"].tensor, offset=(2 * j + half) * D,
                ap=[[0, 64], [1, D]]))
        scratch = small.tile([P, D], F32, tag="fsc")
        nc.vector.scalar_tensor_tensor(
            out=scratch, in0=x_f[j], scalar=float(SCALE_L), in1=tb,
            op0=OP.mult, op1=OP.mult)
        nc.vector.tensor_reduce(out=Lpair[:, j:j + 1], in_=scratch,
                                axis=mybir.AxisListType.X, op=OP.add)
    # scatter pair columns into per-row columns of Lgr
    for par in range(2):
        lg = Lgr[64 * par:64 * par + 64, :].rearrange("p (j two) -> p j two",
                                                      two=2)
        nc.vector.tensor_copy(
            out=lg[:, :, par:par + 1],
            in_=Lpair[64 * par:64 * par + 64, :].rearrange(
                "p (j o) -> p j o", o=1))
    for par in range(2):
        lp = Lgr[64 * par:64 * par + 64, :].rearrange("p (j two) -> p j two",
                                                      two=2)
        nc.vector.tensor_tensor(
            out=lp[:, :, par:par + 1], in0=lp[:, :, par:par + 1],
            in1=pen_tok[64 * par:64 * par + 64, :].rearrange(
                "p (j o) -> p j o", o=1),
            op=OP.add)
    psL = psX.tile([R, P], F32, tag="ctx")
    nc.tensor.transpose(out=psL, in_=Lgr, identity=c_id[:])
    Erg = persist.tile([R, P], F32)
    den = small.tile([R, 1], F32, tag="den")
    nc.scalar.activation(out=Erg, in_=psL, func=AF.Exp, accum_out=den)
    nc.vector.reciprocal(out=den, in_=den)
    nc.vector.tensor_scalar(out=Erg, in0=Erg, scalar1=den, scalar2=None,
                            op0=OP.mult)
    psW = psX.tile([P, R], F32, tag="ctx")
    nc.tensor.transpose(out=psW, in_=Erg, identity=c_id[0:R, 0:R])
    wT = persist.tile([P, R], F32)
    nc.vector.tensor_copy(out=wT, in_=psW)
    for j in range(NXT):
        psO = psX.tile([2, D], F32, tag="ctx")
        nc.tensor.matmul(out=psO, lhsT=wT[:, 2 * j:2 * j + 2],
                         rhs=x_f[j][:], start=True, stop=True)
        o_sb = ev.tile([2, D], F32, tag="osb")
        nc.vector.tensor_copy(out=o_sb, in_=psO)
        nc.sync.dma_start(out=io["out"][2 * j:2 * j + 2, :], in_=o_sb)


# ---------------------------------------------------------------------------
# host side
# ---------------------------------------------------------------------------

_NC_CACHE = {}


def _get_nc():
    if "nc" not in _NC_CACHE:
        nc = bacc.Bacc("TRN2", target_bir_lowering=False, debug=False,
                       enable_asserts=False)
        _build(nc)
        nc.compile()
        _NC_CACHE["nc"] = nc
    return _NC_CACHE["nc"]


def _consts():
    p = np.arange(128)
    iota_g2 = (p % 64).astype(np.float32)[:, None]
    iota_t = np.tile((np.arange(T) + 1.0).astype(np.float32), (128, 1))
    col = np.arange(NXT)
    rowbase = (256.0 * (2 * col[None, :] + p[:, None] // 64)).astype(np.float32)
    ident = np.eye(128, dtype=np.float32)
    identb = np.eye(128, dtype=ml_dtypes.bfloat16)
    iota_gr = np.tile(np.arange(C, dtype=np.float32), (128, 1))
    return dict(iota_g2=iota_g2, iota_t=iota_t, rowbase=rowbase, ident=ident,
                identb=identb, iota_gr=iota_gr)


def _prep_weights(inp):
    wqkv = np.asarray(inp["wqkv"], np.float32)
    bqkv = np.asarray(inp["bqkv"], np.float32)
    wo = np.asarray(inp["wo"], np.float32)
    bo = np.asarray(inp["bo"], np.float32)
    l1g = np.asarray(inp["ln1_g"], np.float32)
    l1b = np.asarray(inp["ln1_b"], np.float32)
    l2g = np.asarray(inp["ln2_g"], np.float32)
    l2b = np.asarray(inp["ln2_b"], np.float32)
    w1 = np.asarray(inp["w1"], np.float32)
    b1 = np.asarray(inp["b1"], np.float32)
    w2 = np.asarray(inp["w2"], np.float32)
    b2 = np.asarray(inp["b2"], np.float32)
    Wq, Wk, Wv = wqkv[:, :D], wqkv[:, D:2 * D], wqkv[:, 2 * D:]
    bq_, bk_, bv_ = bqkv[:, :D], bqkv[:, D:2 * D], bqkv[:, 2 * D:]
    bf = lambda x: np.ascontiguousarray(x.astype(ml_dtypes.bfloat16))
    f32 = lambda x: np.ascontiguousarray(x.astype(np.float32))
    m = {}
    for l in range(L):
        Wqp = Wq[l] * l1g[l][None, :]
        Wkp = Wk[l] * l1g[l][None, :]
        Wvp = Wv[l] * l1g[l][None, :]
        W1p = w1[l] * l2g[l][None, :]
        bqp = Wq[l] @ l1b[l] + bq_[l]
        bkp = Wk[l] @ l1b[l] + bk_[l]
        bvp = Wv[l] @ l1b[l] + bv_[l]
        b1p = w1[l] @ l2b[l] + b1[l]
        bop = wo[l] @ bvp + bo[l]          # v bias folded through wo
        # v layout on chip: head pairs [v0 | 1 | v1 | 1 | v2 | 1 | v3 | 1]
        m[f"wkt{l}"] = bf(Wkp.T)
        m[f"wvt{l}"] = bf(Wvp.T)
        m[f"wqt{l}"] = bf(Wqp.T)
        m[f"wot{l}"] = bf(wo[l].T)
        m[f"w1t{l}"] = bf(W1p.T)
        w2tl = np.empty((128, F), np.float32)
        for fc in range(4):
            w2tl[:, fc * 128:(fc + 1) * 128] = w2[l][:, fc * 128:(fc + 1) * 128].T
        m[f"w2t{l}"] = bf(w2tl)
        m[f"bk{l}"] = f32(bkp[:, None])
        m[f"bq{l}"] = f32(bqp[:, None])
        m[f"bo{l}"] = f32(bop[:, None])
        m[f"b2{l}"] = f32(b2[l][:, None])
        m[f"b1_{l}"] = f32(b1p.reshape(4, 128).T)
    return m


def kernel(**inputs):
    nc = _get_nc()
    wm = _prep_weights(inputs)
    cm = _consts()
    seq = np.asarray(inputs["sequence_item_emb"], np.float32)
    cat = np.asarray(inputs["sequence_cat_ids"])
    msk = np.asarray(inputs["sequence_mask"])
    tgt = np.asarray(inputs["target_item_emb"], np.float32)
    in_maps = []
    for i in range(NCORES):
        rs = slice(i * R, (i + 1) * R)
        im = dict(wm)
        im.update(cm)
        im["seq"] = np.ascontiguousarray(seq[rs].reshape(NT, D))
        im["seqb"] = im["seq"].astype(ml_dtypes.bfloat16)
        im["catm"] = np.ascontiguousarray(
            np.where(msk[rs], cat[rs], -1).astype(np.int8))
        im["tgt"] = np.ascontiguousarray(tgt[rs])
        in_maps.append(im)
    res = run_bass_kernel_spmd(nc, in_maps, list(range(NCORES)))
    _NC_CACHE["last"] = res
    return np.concatenate([res.results[i]["out"] for i in range(NCORES)], axis=0)


# revision 33
# speedup vs baseline: 1.1109x; 1.0274x over previous
"""Trainium2 Bass kernel for nn_CategoryInterestAttention.

Contract: kernel(**inputs) takes FULL unsharded inputs (as produced by the
problem's setup_inputs) and returns the FULL (512, 128) float32 output.

Strategy (pure data parallel, 8 NeuronCores, 64 batch rows each):
  - Categories are indexed by VALUE g in [0, 64) instead of the reference's
    sorted-unique slot index; softmax over present groups is permutation
    invariant so the final output is identical.
  - match[g,t] = (cat[t]==g)&mask[t]; query of group g = seq embedding of the
    LAST matching position (argmax of (t+1)*match), gathered by indirect DMA.
  - LayerNorm gains/biases are folded into the projection weights host-side;
    the normalized sequence z=(seq-mu)/sigma is computed once on-chip and
    shared by both layers. The v bias is folded into the wo bias (attention
    weights sum to 1 per group).
  - bf16 matmuls with fp32 PSUM accumulation; the final target-attention
    stage runs in fp32 (it dominates the error budget).
  - Attention per (row, t-chunk) in a (t, g) layout:
      scoresT  = kT_chunk.T @ q_blocked + (BIG/s)*match  (two matmuls into
                 one PSUM tile; q head-block-diagonal => 4 heads at once)
      E        = exp(s*scoresT - BIG)     (masking folded into the exp bias;
                                           non-matching entries ~1e-13)
      ctx|den  = E_hh.T @ [v_h0|1|v_h1|1] (ones columns give the softmax
                                           denominators for free)
  - All transposes on the PE (DMA transpose measured ~1.2us each on the sync
    engine); attention/FFN outputs are transposed into PSUM and added to the
    fp32 token-major residual directly from PSUM.
"""

import numpy as np
import ml_dtypes

import concourse.bass as bass
import concourse.bacc as bacc
import concourse.tile as tile
from concourse import mybir
from concourse.bass_utils import run_bass_kernel_spmd

F32 = mybir.dt.float32
BF16 = mybir.dt.bfloat16
I32 = mybir.dt.int32
AF = mybir.ActivationFunctionType
OP = mybir.AluOpType

B, T, D = 512, 256, 128
C, H, L, F = 64, 4, 2, 512
HD = D // H                    # 32
NCORES = 8
R = B // NCORES                # 64 rows per core
NT = R * T                     # 16384 seq tokens per core
NX = R * C                     # 4096 group tokens per core
NTC = NT // 128                # 128 seq chunks
NXT = NX // 128                # 32 x-token tiles
SCALE_S = 1.0 / np.sqrt(np.float32(HD))
SCALE_L = 1.0 / np.sqrt(np.float32(D))
EPS = 1e-5
BIG = 30.0                     # mask offset: exp(-30) ~ 9e-14


def _build(nc):
    seq = nc.dram_tensor("seq", [NT, D], F32, kind="ExternalInput").ap()
    seqb = nc.dram_tensor("seqb", [NT, D], BF16, kind="ExternalInput").ap()
    catm = nc.dram_tensor("catm", [R, T], mybir.dt.int8,
                          kind="ExternalInput").ap()
    tgt = nc.dram_tensor("tgt", [R, D], F32, kind="ExternalInput").ap()
    iota_g2 = nc.dram_tensor("iota_g2", [128, 1], F32, kind="ExternalInput").ap()
    iota_t = nc.dram_tensor("iota_t", [128, T], F32, kind="ExternalInput").ap()
    rowbase = nc.dram_tensor("rowbase", [128, NXT], F32, kind="ExternalInput").ap()
    ident = nc.dram_tensor("ident", [128, 128], F32, kind="ExternalInput").ap()
    identb = nc.dram_tensor("identb", [128, 128], BF16, kind="ExternalInput").ap()
    iota_gr = nc.dram_tensor("iota_gr", [128, C], F32, kind="ExternalInput").ap()
    io = dict(seq=seq, seqb=seqb, catm=catm, tgt=tgt, iota_g2=iota_g2,
              iota_t=iota_t, rowbase=rowbase, ident=ident, identb=identb,
              iota_gr=iota_gr)
    for name, shape, dt_ in [
        ("wkt", [D, D], BF16), ("wvt", [D, D], BF16), ("wqt", [D, D], BF16),
        ("wot", [D, D], BF16), ("w1t", [D, F], BF16), ("w2t", [D, F], BF16),
        ("bk", [D, 1], F32), ("bq", [D, 1], F32), ("bo", [D, 1], F32),
        ("b2", [D, 1], F32), ("b1_", [128, 4], F32),
    ]:
        io[name] = [nc.dram_tensor(f"{name}{l}", shape, dt_,
                                   kind="ExternalInput").ap() for l in range(L)]
    io["out"] = nc.dram_tensor("out", [R, D], F32, kind="ExternalOutput").ap()

    with tile.TileContext(nc) as tc:
        from contextlib import ExitStack
        with ExitStack() as ctx:
            _body(ctx, tc, nc, io)
    return nc


def _body(ctx, tc, nc, io):
    P = 128
    persist = ctx.enter_context(tc.tile_pool(name="persist", bufs=1))
    consts = ctx.enter_context(tc.tile_pool(name="consts", bufs=1))
    ld = ctx.enter_context(tc.tile_pool(name="ld", bufs=4))
    small = ctx.enter_context(tc.tile_pool(name="small", bufs=4))
    ev = ctx.enter_context(tc.tile_pool(name="ev", bufs=4))
    ctokp = ctx.enter_context(tc.tile_pool(name="ctokp", bufs=6))
    epool = ctx.enter_context(tc.tile_pool(name="epool", bufs=4))
    psA = ctx.enter_context(tc.tile_pool(name="psA", bufs=3, space="PSUM"))
    psC = ctx.enter_context(tc.tile_pool(name="psC", bufs=3, space="PSUM"))
    psT = ctx.enter_context(tc.tile_pool(name="psT", bufs=2, space="PSUM"))
    psX = psC

    # ---- constants ----
    c_ig2 = consts.tile([P, 1], F32)
    nc.sync.dma_start(out=c_ig2, in_=io["iota_g2"])
    c_it = consts.tile([P, T], F32)
    nc.sync.dma_start(out=c_it, in_=io["iota_t"])
    c_rb = consts.tile([P, NXT], F32)
    nc.sync.dma_start(out=c_rb, in_=io["rowbase"])
    c_id = consts.tile([P, P], F32)
    nc.sync.dma_start(out=c_id, in_=io["ident"])
    c_idb = consts.tile([P, P], BF16)
    nc.sync.dma_start(out=c_idb, in_=io["identb"])
    c_igr = consts.tile([P, C], F32)
    nc.sync.dma_start(out=c_igr, in_=io["iota_gr"])
    c_eps = consts.tile([P, 1], F32)
    nc.vector.memset(c_eps, EPS)
    w = {}
    for name in ("wkt", "wvt", "wqt", "wot", "w1t", "w2t"):
        w[name] = []
        for l in range(L):
            t_ = consts.tile(list(io[name][l].shape), BF16, tag=f"{name}{l}")
            nc.sync.dma_start(out=t_, in_=io[name][l])
            w[name].append(t_)
    bias = {}
    for name in ("bk", "bq", "bo", "b2", "b1_"):
        bias[name] = []
        for l in range(L):
            t_ = consts.tile(list(io[name][l].shape), F32, tag=f"{name}{l}")
            nc.sync.dma_start(out=t_, in_=io[name][l])
            bias[name].append(t_)

    def pe_transpose_b(dst_sbuf_slice, src_tile, ev_engine=None):
        """bf16 (128,128) transpose via PE + evict into an SBUF slice."""
        pt = psT.tile([P, P], BF16, tag="pt")
        nc.tensor.transpose(out=pt, in_=src_tile, identity=c_idb[:])
        (ev_engine or nc.scalar.copy)(out=dst_sbuf_slice, in_=pt)

    # ---- stage 0: z = LN(seq) token-major -> zT feature-major (bf16) ----
    zT = persist.tile([P, NT], BF16)
    for g4 in range(NT // 512):
        s4 = ld.tile([P, 512], BF16, tag="seqld", bufs=2)
        nc.sync.dma_start(out=s4, in_=bass.AP(
            tensor=io["seqb"].tensor, offset=g4 * 512 * D,
            ap=[[D, 128], [128 * D, 4], [1, D]]))
        mvb = small.tile([P, 4, 2], F32, tag="mvb")
        for j in range(4):
            st = small.tile([P, 6], F32, tag="bnst")
            nc.vector.bn_stats(out=st, in_=s4[:, j * 128:(j + 1) * 128])
            nc.vector.bn_aggr(out=mvb[:, j, :], in_=st)
        nc.scalar.activation(out=mvb[:, :, 1:2], in_=mvb[:, :, 1:2],
                             func=AF.Sqrt, bias=c_eps[:])
        nc.vector.reciprocal(out=mvb[:, :, 1:2], in_=mvb[:, :, 1:2])
        z4 = ld.tile([P, 512], BF16, tag="ztok")
        for j in range(4):
            nc.vector.tensor_scalar(
                out=z4[:, j * 128:(j + 1) * 128],
                in0=s4[:, j * 128:(j + 1) * 128], scalar1=mvb[:, j, 0:1],
                scalar2=mvb[:, j, 1:2], op0=OP.subtract, op1=OP.mult)
            pe_transpose_b(zT[:, g4 * 512 + j * 128:g4 * 512 + (j + 1) * 128],
                           z4[:, j * 128:(j + 1) * 128])

    # ---- stage 1: match (g-layout), qidx, present per row-pair ----
    presentf = persist.tile([P, NXT], F32)
    qposf = persist.tile([P, NXT], F32)
    for rp in range(NXT):
        cat_bc = ld.tile([P, T], mybir.dt.int8, tag="catbc", bufs=2)
        for half in range(2):
            r = 2 * rp + half
            nc.gpsimd.dma_start(out=cat_bc[64 * half:64 * half + 64, :],
                                in_=bass.AP(tensor=io["catm"].tensor,
                                            offset=r * T, ap=[[0, 64], [1, T]]))
        catf_bc = small.tile([P, T], F32, tag="catf", bufs=2)
        nc.vector.tensor_copy(out=catf_bc, in_=cat_bc)
        mg = small.tile([P, T], BF16, tag="mg", bufs=2)
        nc.vector.tensor_scalar(out=mg, in0=catf_bc, scalar1=c_ig2,
                                scalar2=None, op0=OP.is_equal)
        nc.vector.tensor_reduce(out=presentf[:, rp:rp + 1], in_=mg,
                                axis=mybir.AxisListType.X, op=OP.max)
        posm = small.tile([P, T], F32, tag="posm", bufs=2)
        nc.vector.tensor_tensor(out=posm, in0=mg, in1=c_it, op=OP.mult)
        nc.vector.tensor_reduce(out=qposf[:, rp:rp + 1], in_=posm,
                                axis=mybir.AxisListType.X, op=OP.max)
    qidx_i = persist.tile([P, NXT], I32)
    tmpq = small.tile([P, NXT], F32, tag="tmpq")
    nc.vector.tensor_scalar(out=tmpq, in0=qposf, scalar1=-1.0, scalar2=0.0,
                            op0=OP.add, op1=OP.max)
    nc.vector.tensor_tensor(out=tmpq, in0=tmpq, in1=c_rb, op=OP.add)
    nc.vector.tensor_copy(out=qidx_i, in_=tmpq)
    pen_tok = persist.tile([P, NXT], F32)
    nc.vector.tensor_scalar(out=pen_tok, in0=presentf, scalar1=-1.0, scalar2=1e9,
                            op0=OP.add, op1=OP.mult)

    # ---- attention match tiles in (t, g) layout, shared by both layers ----
    cat_tok8 = persist.tile([P, NTC], mybir.dt.int8)
    nc.sync.dma_start(out=cat_tok8, in_=bass.AP(
        tensor=io["catm"].tensor, offset=0, ap=[[1, 128], [T, R], [128, 2]]))
    cat_tok = persist.tile([P, NTC], F32)
    nc.vector.tensor_copy(out=cat_tok, in_=cat_tok8)
    m_tg = [persist.tile([P, 2, 1, C], BF16, tag=f"mtg{r}", name=f"mtg{r}")
            for r in range(R)]
    for r in range(R):
        for c in range(2):
            nc.vector.tensor_scalar(out=m_tg[r][:, c, 0, :], in0=c_igr,
                                    scalar1=cat_tok[:, 2 * r + c:2 * r + c + 1],
                                    scalar2=None, op0=OP.is_equal)

    # ---- x0 gather (token-major fp32 master copy of x) ----
    x_f = [persist.tile([P, D], F32, tag=f"x{j}", name=f"x{j}")
           for j in range(NXT)]
    for j in range(NXT):
        nc.gpsimd.indirect_dma_start(
            out=x_f[j][:], out_offset=None, in_=io["seq"][:],
            in_offset=bass.IndirectOffsetOnAxis(ap=qidx_i[:, j:j + 1], axis=0))

    # ---- per-layer persistent buffers ----
    kT = persist.tile([P, NT], BF16)
    # v[cc]: (128, 4, 33) = [v_h | 1] per head (ones column -> denominators)
    v_sb = [persist.tile([P, H, HD + 1], BF16, tag=f"v{cc}", name=f"v{cc}")
            for cc in range(NTC)]
    for cc in range(NTC):
        nc.vector.memset(v_sb[cc][:, :, HD:HD + 1], 1.0)
    xnT = persist.tile([P, NX], BF16)
    # qb slabs: per 8 rows, q in head-block-diagonal layout (zeros elsewhere
    # memset once; the q evictions always overwrite the same block positions)
    qbs = [persist.tile([P, 8 * H * C], BF16, tag=f"qbs{i}", name=f"qbs{i}")
           for i in range(2)]
    for i in range(2):
        nc.vector.memset(qbs[i], 0.0)

    def ln_to(dst_T):
        for g4 in range(NXT // 4):
            mvb = small.tile([P, 4, 2], F32, tag="mvb")
            for j in range(4):
                st = small.tile([P, 6], F32, tag="bnst")
                nc.vector.bn_stats(out=st, in_=x_f[4 * g4 + j])
                nc.vector.bn_aggr(out=mvb[:, j, :], in_=st)
            nc.scalar.activation(out=mvb[:, :, 1:2], in_=mvb[:, :, 1:2],
                                 func=AF.Sqrt, bias=c_eps[:])
            nc.vector.reciprocal(out=mvb[:, :, 1:2], in_=mvb[:, :, 1:2])
            for j in range(4):
                zx = ld.tile([P, D], BF16, tag="zxtok")
                nc.vector.tensor_scalar(out=zx, in0=x_f[4 * g4 + j],
                                        scalar1=mvb[:, j, 0:1],
                                        scalar2=mvb[:, j, 1:2],
                                        op0=OP.subtract, op1=OP.mult)
                pe_transpose_b(dst_T[:, (4 * g4 + j) * 128:(4 * g4 + j + 1) * 128],
                               zx, ev_engine=nc.vector.tensor_copy)

    for l in range(L):
        # ---- kT = Wk' @ z (feature-major), bias via ACT evict ----
        for nn in range(NT // 512):
            ps = psA.tile([P, 512], F32, tag="mm")
            nc.tensor.matmul(out=ps, lhsT=w["wkt"][l][:],
                             rhs=zT[:, nn * 512:(nn + 1) * 512],
                             start=True, stop=True)
            nc.scalar.activation(out=kT[:, nn * 512:(nn + 1) * 512], in_=ps,
                                 func=AF.Identity, bias=bias["bk"][l][:])
        # ---- v token-major, head-pair layout with ones columns ----
        for cc in range(NTC):
            ps = psA.tile([P, D], F32, tag="mm")
            nc.tensor.matmul(out=ps, lhsT=zT[:, cc * 128:(cc + 1) * 128],
                             rhs=w["wvt"][l][:], start=True, stop=True)
            nc.scalar.copy(
                out=v_sb[cc][:, :, 0:HD],
                in_=ps[:].rearrange("p (h c) -> p h c", h=H))
        # ---- attention; 8 rows (one 512-token slab) at a time ----
        ln_to(xnT)
        for sl in range(NX // 512):
            # q for this slab's 8 rows, evicted into block-diagonal layout
            qsl = qbs[sl % 2]
            ps = psA.tile([P, 512], F32, tag="mm")
            nc.tensor.matmul(out=ps, lhsT=w["wqt"][l][:],
                             rhs=xnT[:, sl * 512:(sl + 1) * 512],
                             start=True, stop=True)
            for h in range(H):
                nc.scalar.activation(
                    out=qsl[HD * h:HD * (h + 1), :].rearrange(
                        "p (rl q) -> p rl q", q=H * C)[:, :, C * h:C * (h + 1)],
                    in_=ps[HD * h:HD * (h + 1), :].rearrange(
                        "p (rl g) -> p rl g", g=C),
                    func=AF.Identity, bias=bias["bq"][l][HD * h:HD * (h + 1), :])
            ctx_tok = []
            for rp2 in range(4):           # row pairs within slab
                psc = psC.tile([P, H, HD + 1], F32, tag="ctx")
                for par in range(2):
                    r = 8 * sl + 2 * rp2 + par
                    rl = 2 * rp2 + par
                    qb = qsl[:, rl * H * C:(rl + 1) * H * C]
                    ps = psA.tile([P, 2 * H * C], F32, tag="mm")
                    for c in range(2):
                        cc = 2 * r + c
                        nc.tensor.matmul(out=ps[:, 256 * c:256 * (c + 1)],
                                         lhsT=kT[:, cc * 128:(cc + 1) * 128],
                                         rhs=qb, start=(c == 0), stop=(c == 1))
                    et = epool.tile([P, 2 * H * C], BF16, tag="et")
                    nc.scalar.activation(out=et, in_=ps, func=AF.Exp,
                                         scale=float(SCALE_S))
                    E2 = epool.tile([P, 2 * H * C], BF16, tag="E")
                    nc.vector.tensor_tensor(
                        out=E2[:].rearrange("p (c h g) -> p c h g", c=2, h=H),
                        in0=et[:].rearrange("p (c h g) -> p c h g", c=2, h=H),
                        in1=m_tg[r].to_broadcast([P, 2, H, C]), op=OP.mult)
                    off = 64 * par
                    for h in range(H):
                        for c in range(2):
                            nc.tensor.matmul(
                                out=psc[off:off + 64, h, :],
                                lhsT=E2[:, 256 * c + C * h:256 * c + C * (h + 1)],
                                rhs=v_sb[2 * r + c][:, h, :],
                                start=(h == 0 and c == 0),
                                stop=(h == H - 1 and c == 1))
                rd = small.tile([P, H, 1], F32, tag="rd")
                nc.vector.tensor_scalar(out=rd, in0=psc[:, :, HD:HD + 1],
                                        scalar1=1e-30, scalar2=None, op0=OP.add)
                nc.vector.reciprocal(out=rd, in_=rd)
                ct = ctokp.tile([P, D], BF16, tag="ctok")
                nc.vector.scalar_tensor_tensor(
                    out=ct[:].rearrange("p (h c) -> p h c", h=H),
                    in0=psc[:, :, 0:HD], scalar=1.0,
                    in1=rd.to_broadcast([P, H, HD]),
                    op0=OP.mult, op1=OP.mult)
                ctx_tok.append(ct)
            cT = ev.tile([P, 512], BF16, tag="cT")
            for k in range(4):
                pe_transpose_b(cT[:, k * 128:(k + 1) * 128], ctx_tok[k],
                               ev_engine=nc.vector.tensor_copy)
            ps = psA.tile([P, 512], F32, tag="mm")
            nc.tensor.matmul(out=ps, lhsT=w["wot"][l][:], rhs=cT,
                             start=True, stop=True)
            aoT = ev.tile([P, 512], BF16, tag="aoT")
            nc.scalar.activation(out=aoT, in_=ps, func=AF.Identity,
                                 bias=bias["bo"][l][:])
            for k in range(4):
                j = sl * 4 + k
                pt = psT.tile([P, P], BF16, tag="pt")
                nc.tensor.transpose(out=pt, in_=aoT[:, k * 128:(k + 1) * 128],
                                    identity=c_idb[:])
                nc.vector.tensor_tensor(out=x_f[j], in0=x_f[j], in1=pt,
                                        op=OP.add)

        # ---- FFN ----
        ln_to(xnT)
        for nn in range(NX // 512):
            r1 = []
            for fc in range(4):
                ps = psA.tile([P, 512], F32, tag="mm")
                nc.tensor.matmul(out=ps,
                                 lhsT=w["w1t"][l][:, fc * 128:(fc + 1) * 128],
                                 rhs=xnT[:, nn * 512:(nn + 1) * 512],
                                 start=True, stop=True)
                r1t = ev.tile([P, 512], BF16, tag="r1")
                nc.scalar.activation(out=r1t, in_=ps, func=AF.Relu,
                                     bias=bias["b1_"][l][:, fc:fc + 1])
                r1.append(r1t)
            ps2 = psA.tile([P, 512], F32, tag="mm")
            for fc in range(4):
                nc.tensor.matmul(out=ps2,
                                 lhsT=w["w2t"][l][:, fc * 128:(fc + 1) * 128],
                                 rhs=r1[fc], start=(fc == 0), stop=(fc == 3))
            f2T = ev.tile([P, 512], BF16, tag="aoT")
            nc.scalar.activation(out=f2T, in_=ps2, func=AF.Identity,
                                 bias=bias["b2"][l][:])
            for k in range(4):
                j = nn * 4 + k
                pt = psT.tile([P, P], BF16, tag="pt")
                nc.tensor.transpose(out=pt, in_=f2T[:, k * 128:(k + 1) * 128],
                                    identity=c_idb[:])
                nc.vector.tensor_tensor(out=x_f[j], in0=x_f[j], in1=pt,
                                        op=OP.add)

    # ---- final stage (fp32) ----
    Lgr = persist.tile([P, R], F32)
    nc.vector.memset(Lgr, -1e9)
    Lpair = persist.tile([P, NXT], F32)
    for j in range(NXT):
        tb = ld.tile([P, D], F32, tag="tgtbc", bufs=4)
        for half in range(2):
            eng = nc.sync if half == 0 else nc.gpsimd
            eng.dma_start(out=tb[64 * half:64 * half + 64, :], in_=bass.AP(
                tensor=io["tgt"].tensor, offset=(2 * j + half) * D,
                ap=[[0, 64], [1, D]]))
        scratch = small.tile([P, D], F32, tag="fsc")
        nc.vector.scalar_tensor_tensor(
            out=scratch, in0=x_f[j], scalar=float(SCALE_L), in1=tb,
            op0=OP.mult, op1=OP.mult)
        nc.vector.tensor_reduce(out=Lpair[:, j:j + 1], in_=scratch,
                                axis=mybir.AxisListType.X, op=OP.add)
    # scatter pair columns into per-row columns of Lgr
    for par in range(2):
        lg = Lgr[64 * par:64 * par + 64, :].rearrange("p (j two) -> p j two",
                                                      two=2)
        nc.vector.tensor_copy(
            out=lg[:, :, par:par + 1],
            in_=Lpair[64 * par:64 * par + 64, :].rearrange(
                "p (j o) -> p j o", o=1))
    for par in range(2):
        lp = Lgr[64 * par:64 * par + 64, :].rearrange("p (j two) -> p j two",
                                                      two=2)
        nc.vector.tensor_tensor(
            out=lp[:, :, par:par + 1], in0=lp[:, :, par:par + 1],
            in1=pen_tok[64 * par:64 * par + 64, :].rearrange(
                "p (j o) -> p j o", o=1),
            op=OP.add)
    psL = psX.tile([R, P], F32, tag="ctx")
    nc.tensor.transpose(out=psL, in_=Lgr, identity=c_id[:])
    Erg = persist.tile([R, P], F32)
    den = small.tile([R, 1], F32, tag="den")
    nc.scalar.activation(out=Erg, in_=psL, func=AF.Exp, accum_out=den)
    nc.vector.reciprocal(out=den, in_=den)
    nc.vector.tensor_scalar(out=Erg, in0=Erg, scalar1=den, scalar2=None,
                            op0=OP.mult)
    psW = psX.tile([P, R], F32, tag="ctx")
    nc.tensor.transpose(out=psW, in_=Erg, identity=c_id[0:R, 0:R])
    wT = persist.tile([P, R], F32)
    nc.vector.tensor_copy(out=wT, in_=psW)
    for j in range(NXT):
        psO = psX.tile([2, D], F32, tag="ctx")
        nc.tensor.matmul(out=psO, lhsT=wT[:, 2 * j:2 * j + 2],
                         rhs=x_f[j][:], start=True, stop=True)
        o_sb = ev.tile([2, D], F32, tag="osb")
        nc.vector.tensor_copy(out=o_sb, in_=psO)
        nc.sync.dma_start(out=io["out"][2 * j:2 * j + 2, :], in_=o_sb)


# ---------------------------------------------------------------------------
# host side
# ---------------------------------------------------------------------------

_NC_CACHE = {}


def _get_nc():
    if "nc" not in _NC_CACHE:
        nc = bacc.Bacc("TRN2", target_bir_lowering=False, debug=False,
                       enable_asserts=False)
        _build(nc)
        nc.compile()
        _NC_CACHE["nc"] = nc
    return _NC_CACHE["nc"]


def _consts():
    p = np.arange(128)
    iota_g2 = (p % 64).astype(np.float32)[:, None]
    iota_t = np.tile((np.arange(T) + 1.0).astype(np.float32), (128, 1))
    col = np.arange(NXT)
    rowbase = (256.0 * (2 * col[None, :] + p[:, None] // 64)).astype(np.float32)
    ident = np.eye(128, dtype=np.float32)
    identb = np.eye(128, dtype=ml_dtypes.bfloat16)
    iota_gr = np.tile(np.arange(C, dtype=np.float32), (128, 1))
    return dict(iota_g2=iota_g2, iota_t=iota_t, rowbase=rowbase, ident=ident,
                identb=identb, iota_gr=iota_gr)


def _prep_weights(inp):
    wqkv = np.asarray(inp["wqkv"], np.float32)
    bqkv = np.asarray(inp["bqkv"], np.float32)
    wo = np.asarray(inp["wo"], np.float32)
    bo = np.asarray(inp["bo"], np.float32)
    l1g = np.asarray(inp["ln1_g"], np.float32)
    l1b = np.asarray(inp["ln1_b"], np.float32)
    l2g = np.asarray(inp["ln2_g"], np.float32)
    l2b = np.asarray(inp["ln2_b"], np.float32)
    w1 = np.asarray(inp["w1"], np.float32)
    b1 = np.asarray(inp["b1"], np.float32)
    w2 = np.asarray(inp["w2"], np.float32)
    b2 = np.asarray(inp["b2"], np.float32)
    Wq, Wk, Wv = wqkv[:, :D], wqkv[:, D:2 * D], wqkv[:, 2 * D:]
    bq_, bk_, bv_ = bqkv[:, :D], bqkv[:, D:2 * D], bqkv[:, 2 * D:]
    bf = lambda x: np.ascontiguousarray(x.astype(ml_dtypes.bfloat16))
    f32 = lambda x: np.ascontiguousarray(x.astype(np.float32))
    m = {}
    for l in range(L):
        Wqp = Wq[l] * l1g[l][None, :]
        Wkp = Wk[l] * l1g[l][None, :]
        Wvp = Wv[l] * l1g[l][None, :]
        W1p = w1[l] * l2g[l][None, :]
        bqp = Wq[l] @ l1b[l] + bq_[l]
        bkp = Wk[l] @ l1b[l] + bk_[l]
        bvp = Wv[l] @ l1b[l] + bv_[l]
        b1p = w1[l] @ l2b[l] + b1[l]
        bop = wo[l] @ bvp + bo[l]          # v bias folded through wo
        # v layout on chip: head pairs [v0 | 1 | v1 | 1 | v2 | 1 | v3 | 1]
        m[f"wkt{l}"] = bf(Wkp.T)
        m[f"wvt{l}"] = bf(Wvp.T)
        m[f"wqt{l}"] = bf(Wqp.T)
        m[f"wot{l}"] = bf(wo[l].T)
        m[f"w1t{l}"] = bf(W1p.T)
        w2tl = np.empty((128, F), np.float32)
        for fc in range(4):
            w2tl[:, fc * 128:(fc + 1) * 128] = w2[l][:, fc * 128:(fc + 1) * 128].T
        m[f"w2t{l}"] = bf(w2tl)
        m[f"bk{l}"] = f32(bkp[:, None])
        m[f"bq{l}"] = f32(bqp[:, None])
        m[f"bo{l}"] = f32(bop[:, None])
        m[f"b2{l}"] = f32(b2[l][:, None])
        m[f"b1_{l}"] = f32(b1p.reshape(4, 128).T)
    return m


def kernel(**inputs):
    nc = _get_nc()
    wm = _prep_weights(inputs)
    cm = _consts()
    seq = np.asarray(inputs["sequence_item_emb"], np.float32)
    cat = np.asarray(inputs["sequence_cat_ids"])
    msk = np.asarray(inputs["sequence_mask"])
    tgt = np.asarray(inputs["target_item_emb"], np.float32)
    in_maps = []
    for i in range(NCORES):
        rs = slice(i * R, (i + 1) * R)
        im = dict(wm)
        im.update(cm)
        im["seq"] = np.ascontiguousarray(seq[rs].reshape(NT, D))
        im["seqb"] = im["seq"].astype(ml_dtypes.bfloat16)
        im["catm"] = np.ascontiguousarray(
            np.where(msk[rs], cat[rs], -1).astype(np.int8))
        im["tgt"] = np.ascontiguousarray(tgt[rs])
        in_maps.append(im)
    res = run_bass_kernel_spmd(nc, in_maps, list(range(NCORES)))
    _NC_CACHE["last"] = res
    return np.concatenate([res.results[i]["out"] for i in range(NCORES)], axis=0)
